# revision 1
# baseline (speedup 1.0000x reference)
"""MoE with adaptive gate on 8 trn2 NeuronCores — fp8 DoubleRow version.

Data-parallel over batch (B/8 = 1024 rows per core), feature-major on chip.
The PE floor of the previous fp32r kernel was ~123us; this version runs the
two big GEMMs (L1: x@W1, L3: h2s@W3) as float8e4 DoubleRow matmuls, which
the PE executes at 0.5 cycles/row while contracting two 128-row K-tiles per
instruction (4x fp32r throughput per FLOP).

Plain fp8 is too lossy (6.6e-2 rel err vs the 2e-2 gate), so every fp8
operand is split hi+lo at the SAME scale (lo = e4m3(v - hi), representable
thanks to e4m3 subnormals): a K-tile product (Whi+Wlo)@(xhi+xlo) needs only
3 of the 4 cross terms (lo@lo is ~delta^2, dropped), all accumulating into
ONE psum group with no fixup ops:

  main DR  t2:  (Whi[2t], xhi[2t]) + (Whi[2t+1], xhi[2t+1])   [0.5 DR/K-tile]
  cross DR t:   (Whi[t], xlo[t]) + (Wlo[t], xhi[t])           [1   DR/K-tile]

=> 0.75 cycles/row-equivalent per K-tile vs 1.0 for fp32r, measured end-to-
end rel err ~3e-3. Pair-slot operands are single strided APs via interleaved
layouts: weights [*, t, (hi,lo), M], activations [*, t, (lo,hi), N].

Per-core pipeline (all matmul scales: x*16, W*32 => psum = 512*z):
  gate:  1.5 DR/K-tile on (gw hi/lo, x hi/lo), softmax via exp(z/512+gb),
         ones-matmul partition sum, reciprocal, bcast-matmul (x8 folded in)
  L1:    fp8 DR as above -> silu(psum/512 + b1) -> h1 (f32r)
  L2:    fp32r matmul (K=128 only, DR gains nothing) -> silu -> h2
  gating:h2s8 = h2 * (8*g) via one-hot bcast matmul + DVE mul; hi=e4m3(h2s8)
         (ACT copy), lo = h2s8 - hi (DVE sub, fp8 out)
  L3:    fp8 DR over (e,h) K-tiles, experts paired -> out = psum/256 -> bf16

Output written transposed [D, Bs] in bf16; host transposes/upcasts back.
"""

import sys

sys.path.insert(0, "/opt/trn_rl_repo")

import numpy as np
import ml_dtypes

import concourse.bass as bass
import concourse.tile as tile
from concourse import bacc, mybir
from concourse import bass_utils

B, D, E, H = 8192, 2048, 8, 128
NCORES = 8
Bs = B // NCORES          # batch rows per core
BT = 256                  # DoubleRow moving tile (rhs free = 2*BT = 512)
NBT = Bs // BT            # 4 b-tiles per core
DCH = D // 128            # 16 K-tiles over D
NT2 = DCH // 2            # 8 K-tile pairs
GROUPS = [[0, 1, 2], [3, 4, 5], [6, 7]]

F32 = mybir.dt.float32
F32R = mybir.dt.float32r
F8 = mybir.dt.float8e4
BF16 = mybir.dt.bfloat16
NPF8 = ml_dtypes.float8_e4m3
NPBF16 = ml_dtypes.bfloat16
Silu = mybir.ActivationFunctionType.Silu
Exp = mybir.ActivationFunctionType.Exp
Copy = mybir.ActivationFunctionType.Copy
DR = mybir.MatmulPerfMode.DoubleRow

import os
DELAYS = tuple(int(v) for v in os.environ.get("K_DELAYS", "1,2,5").split(","))
XC_HALVED = int(os.environ.get("K_XCHALF", "0"))
PF_MODE = int(os.environ.get("K_PF", "0"))

S_X = 16.0                # x stored as x*16 in fp8
S_W = 32.0                # weights stored as W*32 in fp8
S_L1 = 1.0 / (S_X * S_W)  # psum of L1/gate = 512 * true value
S_G = 8.0                 # gates folded with x8 for h2s quantization
S_L3 = 1.0 / (S_G * S_W)  # L3 psum = 256 * true value


def _build_module(reps=1):
    nc = bacc.Bacc("TRN2", target_bir_lowering=False, debug=False,
                   num_devices=NCORES)

    # activations interleave (lo, hi) on axis 2; weights (hi, lo) on axis 3/2
    xc = nc.dram_tensor("xc", [NT2, 128, 2, 2, Bs], F8,
                        kind="ExternalInput").ap()
    # gate stationary padded to 16 columns: dual-fp8 LdWeights requires the
    # stationary free width >= 16 (ISA s3_lw_dual_fp8_restrictions)
    gwc = nc.dram_tensor("gwc", [128, DCH, 2, 2 * E], F8,
                         kind="ExternalInput").ap()
    gb = nc.dram_tensor("gb", [E, 1], F32, kind="ExternalInput").ap()
    w1c = nc.dram_tensor("w1c", [E, 128, DCH, 2, H], F8,
                         kind="ExternalInput").ap()
    b1t = nc.dram_tensor("b1t", [H, E], F32, kind="ExternalInput").ap()
    w2 = nc.dram_tensor("w2", [E, H, H], F32R, kind="ExternalInput").ap()
    b2t = nc.dram_tensor("b2t", [H, E], F32, kind="ExternalInput").ap()
    # W3 in 4-dc slabs, partition-major within a slab so one slab DMA is
    # a straight copy into the [128, 4, E, 2, H] SBUF tile
    w3c = nc.dram_tensor("w3c", [DCH // 4, 128, 4, E, 2, H], F8,
                         kind="ExternalInput").ap()
    # oh8[k, e*128 + p] = 8.0 iff k == e; bcasts 8*gates row e across
    # 128 psum partitions via a K=8 matmul.
    oh8 = nc.dram_tensor("oh8", [E, E * 128], F32R, kind="ExternalInput").ap()
    onesd = nc.dram_tensor("onesd", [E, E], F32R, kind="ExternalInput").ap()
    outT = nc.dram_tensor("outT", [D, Bs], BF16, kind="ExternalOutput").ap()
    dbg = {}
    if os.environ.get("K_DEBUG"):
        dbg["expT"] = nc.dram_tensor("d_expT", [E, 512], F32R,
                                     kind="ExternalOutput").ap()
        dbg["gn"] = nc.dram_tensor("d_gn", [E, Bs], F32R,
                                   kind="ExternalOutput").ap()
        dbg["h1"] = nc.dram_tensor("d_h1", [H, Bs], F32R,
                                   kind="ExternalOutput").ap()
        dbg["h2c0"] = nc.dram_tensor("d_h2c0", [128, 2, 2, Bs], F8,
                                     kind="ExternalOutput").ap()
        dbg["y0"] = nc.dram_tensor("d_y0", [128, Bs], BF16,
                                   kind="ExternalOutput").ap()

    with tile.TileContext(nc) as tc:
        with (
            tc.tile_pool(name="persist", bufs=1) as persist,
            tc.tile_pool(name="stream", bufs=2) as stream,
        ):
            # gate weights first on the scalar queue: the first PE work
            # (gate DRs on xc slab 0) needs only gwc + xt0
            gw_sb = persist.tile([128, DCH, 2, 2 * E], F8, tag="gw")
            nc.scalar.dma_start(gw_sb[:], gwc[:])
            ones8 = persist.tile([E, 1], F32R, tag="ones8")
            ones1x8 = persist.tile([1, E], F32R, tag="ones1x8")
            oh_sb = persist.tile([E, E * 128], F32R, tag="oh")
            gb_sb = persist.tile([E, 1], F32, tag="gb")
            b1_sb = persist.tile([H, E], F32, tag="b1")
            b2_sb = persist.tile([H, E], F32, tag="b2")
            w2_sb = persist.tile([H, E, H], F32R, tag="w2")

            def _load_smalls():
                nc.scalar.dma_start(ones8[:], onesd[:, 0:1])
                nc.scalar.dma_start(ones1x8[:], onesd[0:1, :])
                nc.scalar.dma_start(oh_sb[:], oh8[:])
                nc.scalar.dma_start(gb_sb[:], gb[:])
                nc.scalar.dma_start(b1_sb[:], b1t[:])
                nc.scalar.dma_start(b2_sb[:], b2t[:])
                nc.scalar.dma_start(w2_sb[:], w2.rearrange("e h k -> h e k"))

            xt_tiles = [persist.tile([128, 2, 2, Bs], F8, tag="xT", bufs=NT2,
                                     name=f"xt{t2}") for t2 in range(NT2)]
            gn_sb = persist.tile([E, Bs], F32R, tag="gn")
            # h2c pair tiles: [h, expert-in-pair, (lo,hi), b]
            h2c_tiles = [persist.tile([128, 2, 2, Bs], F8, tag="h2c", bufs=4,
                                      name=f"h2c{j}") for j in range(4)]

            for _rep in range(reps):
                with tc.tile_pool(name="psumA", bufs=1, space="PSUM") as psA:
                    # 2 gate psum tiles (16 partitions: 8 real experts +
                    # 8 zero pad), 2 softmax groups per tile
                    pgate = [psA.tile([2 * E, 512], F32, tag="gate", bufs=2,
                                      name=f"pg{i}") for i in range(2)]
                    w1_tiles = {}
                    h1_tiles = {}
                    h2t = {}

                    def _w1_alloc(e):
                        w1_tiles[e] = stream.tile([128, DCH, 2, H], F8,
                                                  tag="w1", bufs=5,
                                                  name=f"w1_{e}")

                    def _w1_load_half(e, hv):
                        if hv == 0:
                            _w1_alloc(e)
                            nc.scalar.dma_start(w1_tiles[e][:, :DCH // 2],
                                                w1c[e][:, :DCH // 2])
                        else:
                            nc.scalar.dma_start(w1_tiles[e][:, DCH // 2:],
                                                w1c[e][:, DCH // 2:])

                    def _w1_load(e, halves=False):
                        if halves:
                            _w1_load_half(e, 0)
                            _w1_load_half(e, 1)
                        else:
                            _w1_alloc(e)
                            nc.scalar.dma_start(w1_tiles[e][:], w1c[e])

                    def _l1_drs(ph, wt, t2, bt, start, stop):
                        """3 DRs for K-tile pair t2 into psum slice for bt.

                        One psum bank (2KB zero region) holds two bt slices:
                        start only on the bank's first matmul (bt even),
                        stop only on its last (bt odd).
                        """
                        po = ph[bt // 2][:, (bt % 2) * BT:(bt % 2 + 1) * BT]
                        xt = xt_tiles[t2]
                        bs = bass.ts(bt, BT)
                        nc.tensor.matmul(po, wt[:, 2 * t2:2 * t2 + 2, 0, :],
                                         xt[:, :, 1, bs],
                                         start=(start and bt % 2 == 0),
                                         stop=False, perf_mode=DR)
                        nc.tensor.matmul(po, wt[:, 2 * t2, :, :],
                                         xt[:, 0, :, bs], start=False,
                                         stop=False, perf_mode=DR)
                        nc.tensor.matmul(po, wt[:, 2 * t2 + 1, :, :],
                                         xt[:, 1, :, bs], start=False,
                                         stop=(stop and bt % 2 == 1),
                                         perf_mode=DR)

                    def _silu_h1(e, ph1_e):
                        h1_tiles[e] = stream.tile([H, Bs], F32R, tag="h1",
                                                  bufs=4, name=f"h1_{e}")
                        for i in range(2):
                            bs = bass.ts(i, 512)
                            nc.scalar.activation(h1_tiles[e][:, bs],
                                                 ph1_e[i][:], Silu,
                                                 bias=b1_sb[:, e:e + 1],
                                                 scale=S_L1)

                    for gi, grp in enumerate(GROUPS):
                        ph1 = {}
                        for e in grp:
                            # group 1's first expert rides the gate banks
                            # (idle after softmax): the acc rotation keeps
                            # two spare slots through group 0's L2 block
                            tg, nb = ("gate", 2) if (gi >= 1 and e == grp[0]) \
                                else ("acc", 6)
                            ph1[e] = [psA.tile([128, 512], F32, tag=tg,
                                               bufs=nb, name=f"ph1_{e}_{i}")
                                      for i in range(2)]

                        # delayed experts start late (their W1 DMA is
                        # staggered so xc keeps streaming) and catch up on
                        # the last K-tile pairs after the loop; expert e
                        # processes pairs in arrival order 0,1,2,...
                        if gi == 0:
                            delay = {e: DELAYS[i] if i < len(DELAYS) else
                                     2 * i + 1 for i, e in enumerate(grp)}
                        else:
                            delay = {e: 0 for e in grp}

                        nxt = GROUPS[gi + 1] if gi + 1 < len(GROUPS) else []
                        prefetch_at = {}
                        if gi == 0:
                            # own group staggered in halves, next group's
                            # loads late in the step loop
                            for i, e in enumerate(grp[1:]):
                                s0 = 2 * i + 1
                                prefetch_at.setdefault(s0, []).extend(
                                    [(e, 0), (e, 1)])
                            for i, e in enumerate(nxt):
                                prefetch_at.setdefault(
                                    NT2 - len(nxt) + i, []).append((e, None))
                        else:
                            for i, e in enumerate(nxt):
                                prefetch_at[1 + i] = [(e, None)]

                        for step in range(NT2):
                            t2 = step
                            if gi == 0 and _rep == 0:
                                if XC_HALVED:
                                    # b-halves: bt0/1 DRs can start after
                                    # half the slab has landed
                                    nc.sync.dma_start(
                                        xt_tiles[t2][:, :, :, :512],
                                        xc[t2][:, :, :, :512])
                                    nc.sync.dma_start(
                                        xt_tiles[t2][:, :, :, 512:],
                                        xc[t2][:, :, :, 512:])
                                else:
                                    nc.sync.dma_start(xt_tiles[t2][:], xc[t2])
                            if gi == 0 and step == 0:
                                # issued after xt0 so x wins the DMA race
                                _w1_load(grp[0], halves=True)
                            for pe_, hv in prefetch_at.get(step, []):
                                if hv is None:
                                    _w1_load(pe_)
                                else:
                                    _w1_load_half(pe_, hv)
                            if gi == 0 and step == 4:
                                _load_smalls()
                            if gi == 0:
                                for bt in range(NBT):
                                    po = pgate[bt // 2][:, (bt % 2) * BT:
                                                        (bt % 2 + 1) * BT]
                                    xt = xt_tiles[t2]
                                    bs = bass.ts(bt, BT)
                                    nc.tensor.matmul(
                                        po, gw_sb[:, 2 * t2:2 * t2 + 2, 0, :],
                                        xt[:, :, 1, bs],
                                        start=(t2 == 0 and bt % 2 == 0),
                                        stop=False, perf_mode=DR)
                                    nc.tensor.matmul(
                                        po, gw_sb[:, 2 * t2, :, :],
                                        xt[:, 0, :, bs], start=False,
                                        stop=False, perf_mode=DR)
                                    nc.tensor.matmul(
                                        po, gw_sb[:, 2 * t2 + 1, :, :],
                                        xt[:, 1, :, bs], start=False,
                                        stop=(t2 == NT2 - 1 and bt % 2 == 1),
                                        perf_mode=DR)
                            for e in grp:
                                if step < delay[e]:
                                    continue
                                ct2 = step - delay[e]
                                for bt in range(NBT):
                                    _l1_drs(ph1[e], w1_tiles[e], ct2, bt,
                                            start=(step == delay[e]),
                                            stop=(step == NT2 - 1
                                                  and delay[e] == 0))

                        if gi == 0:
                            # softmax stage 1 issued before the catch-up so
                            # the ACT exp latency hides under catch-up DRs
                            expT, zt, recip, pr8 = {}, {}, {}, {}
                            for i in range(2):
                                expT[i] = stream.tile([E, 512], F32R, tag="expT",
                                                      bufs=2, name=f"expT{i}")
                                nc.scalar.activation(expT[i][:],
                                                     pgate[i][0:E, :],
                                                     Exp, bias=gb_sb[:],
                                                     scale=S_L1)
                            # catch-up K-tile pairs skipped while W1 was in
                            # flight (same accumulation groups); each
                            # expert's silu issues as soon as it closes
                            for e in grp:
                                d = delay[e]
                                for j, ct2 in enumerate(range(NT2 - d, NT2)):
                                    for bt in range(NBT):
                                        _l1_drs(ph1[e], w1_tiles[e], ct2, bt,
                                                start=False, stop=(j == d - 1))
                                _silu_h1(e, ph1[e])
                                if dbg and e == grp[0]:
                                    nc.sync.dma_start(dbg["h1"],
                                                      h1_tiles[e][:])
                            # softmax: gn[e, b] = exp(z/512+gb) / sum_e
                            # (stage-wise so the gate psum slot rotation
                            # never reuses a bank before its exp read)
                            for i in range(2):
                                zt[i] = psA.tile([E, 512], F32, tag="gate",
                                                 bufs=2, name=f"zt{i}")
                                nc.tensor.matmul(zt[i][0:1, :], ones8[:],
                                                 expT[i][:], start=True,
                                                 stop=True)
                            for i in range(2):
                                recip[i] = stream.tile([1, 512], F32R, tag="recip",
                                                       bufs=2, name=f"recip{i}")
                                with nc.allow_low_precision(
                                        reason="f32r softmax denom"):
                                    nc.vector.reciprocal(recip[i][:],
                                                         zt[i][0:1, :])
                            for i in range(2):
                                pr8[i] = psA.tile([E, 512], F32, tag="gate",
                                                  bufs=2, name=f"pr8_{i}")
                                nc.tensor.matmul(pr8[i][:], ones1x8[:],
                                                 recip[i][:], start=True,
                                                 stop=True)
                            for i in range(2):
                                nc.vector.tensor_mul(gn_sb[:, bass.ts(i, 512)],
                                                     expT[i][:], pr8[i][:])
                            if dbg:
                                nc.sync.dma_start(dbg["expT"], expT[0][:])
                                nc.sync.dma_start(dbg["gn"], gn_sb[:])
                        else:
                            for e in grp:
                                _silu_h1(e, ph1[e])

                        if gi == len(GROUPS) - 1:
                            # W3 streams in 4-dc slabs on the sync queue so
                            # the issue never serializes behind the scalar
                            # engine's activation backlog
                            w3_slabs = []
                            for sj in range(3):
                                w3s = stream.tile([128, 4, E, 2, H], F8,
                                                  tag="w3q", bufs=3,
                                                  name=f"w3s{sj}")
                                nc.sync.dma_start(w3s[:], w3c[sj])
                                w3_slabs.append(w3s)

                        # L2 + gating + h2 quantization, batched by stage
                        pgb = {}
                        ph2 = {}
                        for e in grp:
                            for i in range(2):
                                bs = bass.ts(i, 512)
                                ph2[e, i] = psA.tile([H, 512], F32, tag="acc",
                                                     bufs=6,
                                                     name=f"ph2_{e}_{i}")
                                nc.tensor.matmul(ph2[e, i][:], w2_sb[:, e, :],
                                                 h1_tiles[e][:, bs],
                                                 start=True, stop=True)
                        for e in grp:
                            for i in range(2):
                                h2t[e, i] = stream.tile([H, 512], F32,
                                                        tag="h2t", bufs=6,
                                                        name=f"h2t_{e}_{i}")
                                nc.scalar.activation(h2t[e, i][:],
                                                     ph2[e, i][:], Silu,
                                                     bias=b2_sb[:, e:e + 1],
                                                     scale=1.0)
                        for e in grp:
                            for i in range(2):
                                bs = bass.ts(i, 512)
                                pgb[e, i] = psA.tile([128, 512], F32,
                                                     tag="acc", bufs=6,
                                                     name=f"pgb_{e}_{i}")
                                nc.tensor.matmul(pgb[e, i][:],
                                                 oh_sb[:, e * 128:
                                                       (e + 1) * 128],
                                                 gn_sb[:, bs],
                                                 start=True, stop=True)
                        for e in grp:
                            hc = h2c_tiles[e // 2]
                            ei = e % 2
                            for i in range(2):
                                bs = bass.ts(i, 512)
                                h2s8 = stream.tile([H, 512], F32, tag="h2s8",
                                                   bufs=4, name=f"h2s8_{e}_{i}")
                                with nc.allow_low_precision(
                                        reason="fp8 hi/lo split"):
                                    nc.vector.tensor_mul(h2s8[:], h2t[e, i][:],
                                                         pgb[e, i][:])
                                    nc.scalar.activation(hc[:, ei, 1, bs],
                                                         h2s8[:], Copy)
                                    nc.vector.tensor_sub(hc[:, ei, 0, bs],
                                                         h2s8[:],
                                                         hc[:, ei, 1, bs])

                    if dbg:
                        nc.sync.dma_start(dbg["h2c0"], h2c_tiles[0][:])
                    # ---- L3: outT[dc] = sum_e W3[e,dc].T @ h2s[e] ----
                    # Experts 0-5 are emitted before 6-7 in each tile so the
                    # PE can start L3 while the last group's h2 quantization
                    # chain (experts 6/7) is still draining on ACT/DVE.
                    # po tiles come from the same "acc" rotation as phase A
                    # (one shared psum pool -> no pool-close barrier).
                    for dc in range(DCH):
                        if dc == 4:
                            w3s = stream.tile([128, 4, E, 2, H], F8,
                                              tag="w3q", bufs=3, name="w3s3")
                            nc.sync.dma_start(w3s[:], w3c[3])
                            w3_slabs.append(w3s)
                        w3t = w3_slabs[dc // 4]
                        dm = dc % 4
                        for half in range(2):
                            # dc0 rides the gate banks (idle since softmax)
                            # so L3 psum never waits on the last group's
                            # gating chain through the acc rotation
                            if dc == 0:
                                po = psA.tile([128, 512], F32, tag="gate",
                                              bufs=2, name=f"pog{half}")
                            else:
                                po = psA.tile([128, 512], F32, tag="acc",
                                              bufs=6,
                                              name=f"po{(dc * 2 + half) % 6}")
                            for sub in range(2):
                                bt = half * 2 + sub
                                bs = bass.ts(bt, BT)
                                pslice = po[:, sub * BT:(sub + 1) * BT]
                                for j in range(3):
                                    nc.tensor.matmul(
                                        pslice,
                                        w3t[:, dm, 2 * j:2 * j + 2, 0, :],
                                        h2c_tiles[j][:, :, 1, bs],
                                        start=(j == 0 and sub == 0),
                                        stop=False, perf_mode=DR)
                                for e in range(6):
                                    nc.tensor.matmul(
                                        pslice, w3t[:, dm, e, :, :],
                                        h2c_tiles[e // 2][:, e % 2, :, bs],
                                        start=False, stop=False,
                                        perf_mode=DR)
                            for sub in range(2):
                                bt = half * 2 + sub
                                bs = bass.ts(bt, BT)
                                pslice = po[:, sub * BT:(sub + 1) * BT]
                                nc.tensor.matmul(
                                    pslice, w3t[:, dm, 6:8, 0, :],
                                    h2c_tiles[3][:, :, 1, bs],
                                    start=False, stop=False, perf_mode=DR)
                                for e in (6, 7):
                                    nc.tensor.matmul(
                                        pslice, w3t[:, dm, e, :, :],
                                        h2c_tiles[3][:, e % 2, :, bs],
                                        start=False,
                                        stop=(e == 7 and sub == 1),
                                        perf_mode=DR)
                            # copy into the per-dc output staging tile;
                            # one DMA per dc (HWDGE descriptor-gen is the
                            # scarce resource, not bandwidth)
                            with nc.allow_low_precision(
                                    reason="bf16 output"):
                                if dc == DCH - 1:
                                    # tail: one engine per half (a split
                                    # within one tile serializes on the
                                    # tile's write tracking), DMA fired
                                    # immediately per half
                                    o_tl = stream.tile([128, 512], BF16,
                                                       tag="osbt", bufs=2,
                                                       name=f"ot{half}")
                                    if half == 0:
                                        nc.vector.tensor_scalar_mul(
                                            o_tl[:], po[:], S_L3)
                                    else:
                                        nc.scalar.activation(
                                            o_tl[:], po[:], Copy, scale=S_L3)
                                    nc.sync.dma_start(
                                        outT[dc * 128:(dc + 1) * 128,
                                             bass.ts(half, 512)], o_tl[:])
                                    continue
                                if half == 0:
                                    o_sb = stream.tile([128, Bs], BF16,
                                                       tag="osb", bufs=3,
                                                       name=f"osb{dc % 3}")
                                hs = bass.ts(half, 512)
                                if half == 0:
                                    nc.scalar.activation(o_sb[:, hs], po[:],
                                                         Copy, scale=S_L3)
                                else:
                                    nc.vector.tensor_scalar_mul(
                                        o_sb[:, hs], po[:], S_L3)
                            if half == 1:
                                nc.sync.dma_start(
                                    outT[dc * 128:(dc + 1) * 128, :],
                                    o_sb[:])
                                if dbg and dc == 0:
                                    nc.sync.dma_start(dbg["y0"], o_sb[:])

    nc.compile()
    return nc


_MODULE_CACHE = {}


def _get_module(reps=1):
    if reps not in _MODULE_CACHE:
        _MODULE_CACHE[reps] = _build_module(reps)
    return _MODULE_CACHE[reps]


def _hilo(a, scale):
    """Return (hi, lo) e4m3 arrays for a*scale, lo at the SAME scale."""
    s = np.float32(scale)
    hi = (a * s).astype(NPF8)
    lo = (a * s - hi.astype(np.float32)).astype(NPF8)
    return hi, lo


def _prep_shared(gate_w, gate_b, W1, b1, W2, b2, W3):
    # gate weights: [128, DCH, 2, E], pairs (hi, lo)
    gwr = np.zeros((128, DCH, 2 * E), dtype=np.float32)
    gwr[:, :, :E] = gate_w.reshape(DCH, 128, E).transpose(1, 0, 2)
    ghi, glo = _hilo(gwr, S_W)
    gwc = np.ascontiguousarray(np.stack([ghi, glo], axis=2))

    # W1: [E, 128, DCH, 2, H], pairs (hi, lo)
    w1r = W1.reshape(E, DCH, 128, H).transpose(0, 2, 1, 3)
    w1hi, w1lo = _hilo(w1r, S_W)
    w1c = np.ascontiguousarray(np.stack([w1hi, w1lo], axis=3))

    # W3: [DCH, 128, E, 2, H]  (partition = d-chunk rows? no: partition is
    # the h contraction dim; W3[e] is [H, D]: lhsT per dc = [128(h), 128(d)])
    w3r = W3.reshape(E, H, DCH, 128).transpose(2, 1, 0, 3)
    # w3r: [DCH, H(128 partitions), E, 128(d cols)]
    w3hi, w3lo = _hilo(w3r, S_W)
    w3c = np.stack([w3hi, w3lo], axis=3)          # [DCH, 128, E, 2, 128]
    w3c = np.ascontiguousarray(
        w3c.reshape(4, 4, 128, E, 2, 128).transpose(0, 2, 1, 3, 4, 5))

    oh8 = np.zeros((E, E * 128), dtype=np.float32)
    for e in range(E):
        oh8[e, e * 128:(e + 1) * 128] = S_G
    return {
        "gwc": gwc, "gb": np.ascontiguousarray(gate_b.reshape(E, 1)),
        "w1c": w1c, "b1t": np.ascontiguousarray(b1.T),
        "w2": np.ascontiguousarray(W2), "b2t": np.ascontiguousarray(b2.T),
        "w3c": w3c, "oh8": oh8,
        "onesd": np.ones((E, E), dtype=np.float32),
    }


def _prep_xc(x_slice):
    """x slice [Bs, D] -> [NT2, 128, 2, 2, Bs] fp8, pairs (lo, hi)."""
    xT = x_slice.T.reshape(DCH, 128, Bs)
    xhi, xlo = _hilo(xT, S_X)
    # [DCH, 128, 2(lo,hi), Bs] -> [NT2, 2, 128, 2, Bs] -> [NT2, 128, 2, 2, Bs]
    st = np.stack([xlo, xhi], axis=2).reshape(NT2, 2, 128, 2, Bs)
    return np.ascontiguousarray(st.transpose(0, 2, 1, 3, 4))


def kernel(x, gate_w, gate_b, W1, b1, W2, b2, W3, b3):
    x = np.asarray(x, dtype=np.float32)
    gate_w = np.asarray(gate_w, dtype=np.float32)
    gate_b = np.asarray(gate_b, dtype=np.float32)
    W1 = np.asarray(W1, dtype=np.float32)
    b1 = np.asarray(b1, dtype=np.float32)
    W2 = np.asarray(W2, dtype=np.float32)
    b2 = np.asarray(b2, dtype=np.float32)
    W3 = np.asarray(W3, dtype=np.float32)
    b3 = np.asarray(b3, dtype=np.float32)

    nc = _get_module(1)
    shared = _prep_shared(gate_w, gate_b, W1, b1, W2, b2, W3)
    in_maps = [{"xc": _prep_xc(x[i * Bs:(i + 1) * Bs, :]), **shared}
               for i in range(NCORES)]
    try:
        res = bass_utils.run_bass_kernel_spmd(
            nc, in_maps, core_ids=list(range(NCORES)))
    except Exception:
        # the axon-tunneled devices occasionally report a transient
        # NRT_EXEC_UNIT_UNRECOVERABLE; one retry after a pause clears it
        import time as _time
        _time.sleep(30)
        res = bass_utils.run_bass_kernel_spmd(
            nc, in_maps, core_ids=list(range(NCORES)))

    out = np.empty((B, D), dtype=np.float32)
    for i in range(NCORES):
        out[i * Bs:(i + 1) * Bs, :] = res.results[i]["outT"].T.astype(
            np.float32)

    if np.any(b3):
        # b3 contributes sum_e gates[b,e] * b3[e,d]; the device kernel skips
        # it (it is zero for this problem's inputs), so patch on host.
        logits = x @ gate_w + gate_b
        m = logits.max(axis=1, keepdims=True)
        p = np.exp(logits - m)
        gates = p / p.sum(axis=1, keepdims=True)
        out += gates @ b3
    return out



# revision 11
# speedup vs baseline: 1.1056x; 1.1056x over previous
"""MoE with adaptive gate on 8 trn2 NeuronCores — fp8 DoubleRow, partial W-corr.

Data-parallel over batch (B/8 = 1024 rows per core), feature-major on chip.
The two big GEMMs (L1: x@W1, L3: h2s@W3) run as float8e4 DoubleRow matmuls
(0.5 cycles/row, two 128-row K-tiles per instruction).

Every fp8 operand is split hi+lo at the SAME scale (lo = e4m3(v - hi)); a
K-tile product (Whi+Wlo)@(xhi+xlo) needs 3 of the 4 cross terms (lo@lo is
~delta^2, dropped).  This version additionally drops the W-correction term
(Wlo@xhi) on L1 K-tile pairs 2..7, keeping it only on pairs 0..1: measured
end-to-end rel err 1.65e-2 vs the 2e-2 gate (numpy-emulated, emulator matches
device to 4 digits).  Per K-tile-pair DR count: pairs 0-1: 3, pairs 2-7: 2.

  main DR   t2: (Whi[2t], xhi[2t]) + (Whi[2t+1], xhi[2t+1])
  x-corr DR t2: (Whi[2t], xlo[2t]) + (Whi[2t+1], xlo[2t+1])
  W-corr DR t2 (t2<2): (Wlo[2t], xhi[2t]) + (Wlo[2t+1], xhi[2t+1])

The gate keeps all 3 terms (its logit error amplifies through softmax), and
L3 keeps all 3 terms per expert.

Softmax helpers run off the PE: the expert-sum uses a gpsimd (Pool engine)
partition_all_reduce, the per-expert gate row broadcast to 128 partitions
uses gpsimd partition_broadcast (PE one-hot matmuls removed).

Per-core pipeline (all matmul scales: x*16, W*32 => psum = 512*z):
  gate:  1.5 DR/K-tile on (gw hi/lo, x hi/lo), softmax via exp(z/512+gb),
         Pool allreduce, DVE reciprocal * 8, DVE mul -> gn8 = 8*gates
  L1:    fp8 DR as above -> silu(psum/512 + b1) -> h1 (f32r)
  L2:    fp32r matmul (K=128 only, DR gains nothing) -> silu -> h2
  gating:h2s8 = h2 * pgb (Pool bcast of gn8 row) via DVE mul; hi=e4m3(h2s8)
         (ACT copy), lo = h2s8 - hi (DVE sub, fp8 out)
  L3:    fp8 DR over (e,h) K-tiles, experts paired -> out = psum/256 -> bf16

Output written transposed [D, Bs] in bf16; host transposes/upcasts back.
"""

import sys

sys.path.insert(0, "/opt/trn_rl_repo")

import numpy as np
import ml_dtypes

import concourse.bass as bass
import concourse.tile as tile
from concourse import bacc, mybir
from concourse import bass_utils
from concourse import bass_isa

B, D, E, H = 8192, 2048, 8, 128
NCORES = 8
Bs = B // NCORES          # batch rows per core
BT = 256                  # DoubleRow moving tile (rhs free = 2*BT = 512)
NBT = Bs // BT            # 4 b-tiles per core
DCH = D // 128            # 16 K-tiles over D
NT2 = DCH // 2            # 8 K-tile pairs
KW1 = 2                   # K-tile pairs with the L1 W-corr term kept
GROUPS = [[0, 1, 2], [3, 4, 5], [6, 7]]

F32 = mybir.dt.float32
F32R = mybir.dt.float32r
F8 = mybir.dt.float8e4
BF16 = mybir.dt.bfloat16
NPF8 = ml_dtypes.float8_e4m3
NPBF16 = ml_dtypes.bfloat16
Silu = mybir.ActivationFunctionType.Silu
Exp = mybir.ActivationFunctionType.Exp
Copy = mybir.ActivationFunctionType.Copy
DR = mybir.MatmulPerfMode.DoubleRow

import os
DELAYS = tuple(int(v) for v in os.environ.get("K_DELAYS", "1,2,5").split(","))
XC_HALVED = int(os.environ.get("K_XCHALF", "0"))

S_X = 16.0                # x stored as x*16 in fp8
S_W = 32.0                # weights stored as W*32 in fp8
S_L1 = 1.0 / (S_X * S_W)  # psum of L1/gate = 512 * true value
S_G = 8.0                 # gates folded with x8 for h2s quantization
S_L3 = 1.0 / (S_G * S_W)  # L3 psum = 256 * true value


def _build_module(reps=1):
    nc = bacc.Bacc("TRN2", target_bir_lowering=False, debug=False,
                   num_devices=NCORES)

    # activations interleave (lo, hi) on axis 2; weights (hi, lo) on axis 3/2
    xc = nc.dram_tensor("xc", [NT2, 128, 2, 2, Bs], F8,
                        kind="ExternalInput").ap()
    # gate stationary padded to 16 columns: dual-fp8 LdWeights requires the
    # stationary free width >= 16 (ISA s3_lw_dual_fp8_restrictions)
    gwc = nc.dram_tensor("gwc", [128, DCH, 2, 2 * E], F8,
                         kind="ExternalInput").ap()
    gb = nc.dram_tensor("gb", [E, 1], F32, kind="ExternalInput").ap()
    # W1 in two contiguous regions: A = K-tile pairs 0..KW1-1 with hi+lo
    # planes (W-corr kept there), B = remaining tiles hi plane only
    w1a = nc.dram_tensor("w1a", [E, 128, 2 * KW1, 2, H], F8,
                         kind="ExternalInput").ap()
    w1b = nc.dram_tensor("w1b", [E, 128, DCH - 2 * KW1, H], F8,
                         kind="ExternalInput").ap()
    b1t = nc.dram_tensor("b1t", [H, E], F32, kind="ExternalInput").ap()
    w2 = nc.dram_tensor("w2", [E, H, H], F32R, kind="ExternalInput").ap()
    b2t = nc.dram_tensor("b2t", [H, E], F32, kind="ExternalInput").ap()
    # W3 in 4-dc slabs, partition-major within a slab so one slab DMA is
    # a straight copy into the [128, 4, E, 2, H] SBUF tile
    w3c = nc.dram_tensor("w3c", [DCH // 4, 128, 4, E, 2, H], F8,
                         kind="ExternalInput").ap()
    outT = nc.dram_tensor("outT", [D, Bs], BF16, kind="ExternalOutput").ap()
    dbg = {}
    if os.environ.get("K_DEBUG"):
        dbg["expT"] = nc.dram_tensor("d_expT", [E, 512], F32R,
                                     kind="ExternalOutput").ap()
        dbg["gn"] = nc.dram_tensor("d_gn", [E, Bs], F32R,
                                   kind="ExternalOutput").ap()
        dbg["h1"] = nc.dram_tensor("d_h1", [H, Bs], F32R,
                                   kind="ExternalOutput").ap()
        dbg["h2c0"] = nc.dram_tensor("d_h2c0", [128, 2, 2, Bs], F8,
                                     kind="ExternalOutput").ap()
        dbg["y0"] = nc.dram_tensor("d_y0", [128, Bs], BF16,
                                   kind="ExternalOutput").ap()

    with tile.TileContext(nc) as tc:
        with (
            tc.tile_pool(name="persist", bufs=1) as persist,
            tc.tile_pool(name="stream", bufs=2) as stream,
        ):
            # gate weights first on the sync queue: the first PE work
            # (gate DRs on xc slab 0) needs only gwc + xt0
            gw_sb = persist.tile([128, DCH, 2, 2 * E], F8, tag="gw")
            nc.sync.dma_start(gw_sb[:], gwc[:])
            gb_sb = persist.tile([E, 1], F32, tag="gb")
            b1_sb = persist.tile([H, E], F32, tag="b1")
            b2_sb = persist.tile([H, E], F32, tag="b2")
            w2_sb = persist.tile([H, E, H], F32R, tag="w2")

            def _load_smalls():
                nc.scalar.dma_start(gb_sb[:], gb[:])
                nc.scalar.dma_start(b1_sb[:], b1t[:])
                nc.scalar.dma_start(b2_sb[:], b2t[:])
                nc.scalar.dma_start(w2_sb[:], w2.rearrange("e h k -> h e k"))

            xt_tiles = [persist.tile([128, 2, 2, Bs], F8, tag="xT", bufs=NT2,
                                     name=f"xt{t2}") for t2 in range(NT2)]
            gn_sb = persist.tile([E, Bs], F32R, tag="gn")
            # h2c pair tiles: [h, expert-in-pair, (lo,hi), b]
            h2c_tiles = [persist.tile([128, 2, 2, Bs], F8, tag="h2c", bufs=4,
                                      name=f"h2c{j}") for j in range(4)]

            for _rep in range(reps):
                with tc.tile_pool(name="psumA", bufs=1, space="PSUM") as psA:
                    # 2 gate psum tiles (16 partitions: 8 real experts +
                    # 8 zero pad), 2 softmax groups per tile
                    pgate = [psA.tile([2 * E, 512], F32, tag="gate", bufs=2,
                                      name=f"pg{i}") for i in range(2)]
                    w1a_tiles = {}
                    w1b_tiles = {}
                    h1_tiles = {}
                    h2t = {}
                    pgb_tiles = {}

                    def _w1_load_half(e, hv):
                        # half 0: K-tile pairs 0..KW1-1 both planes (hi+lo,
                        # W-corr kept there); half 1: remaining tiles hi only
                        if hv == 0:
                            w1a_tiles[e] = stream.tile(
                                [128, 2 * KW1, 2, H], F8, tag="w1a", bufs=5,
                                name=f"w1a_{e}")
                            nc.scalar.dma_start(w1a_tiles[e][:], w1a[e])
                        else:
                            w1b_tiles[e] = stream.tile(
                                [128, DCH - 2 * KW1, H], F8, tag="w1b",
                                bufs=5, name=f"w1b_{e}")
                            nc.scalar.dma_start(w1b_tiles[e][:], w1b[e])

                    def _w1_load(e, halves=False):
                        _w1_load_half(e, 0)
                        _w1_load_half(e, 1)

                    def _l1_drs(e, ph, t2, bt, start, stop):
                        """2-3 DRs for K-tile pair t2 into psum slice for bt.

                        One psum bank (2KB zero region) holds two bt slices:
                        start only on the bank's first matmul (bt even),
                        stop only on its last (bt odd).
                        """
                        po = ph[bt // 2][:, (bt % 2) * BT:(bt % 2 + 1) * BT]
                        xt = xt_tiles[t2]
                        bs = bass.ts(bt, BT)
                        wcorr = t2 < KW1
                        if wcorr:
                            whi = w1a_tiles[e][:, 2 * t2:2 * t2 + 2, 0, :]
                        else:
                            t0 = 2 * (t2 - KW1)
                            whi = w1b_tiles[e][:, t0:t0 + 2, :]
                        # main: (Whi pair) @ (xhi pair)
                        nc.tensor.matmul(po, whi, xt[:, :, 1, bs],
                                         start=(start and bt % 2 == 0),
                                         stop=False, perf_mode=DR)
                        # x-corr: (Whi pair) @ (xlo pair)
                        nc.tensor.matmul(po, whi, xt[:, :, 0, bs],
                                         start=False,
                                         stop=(stop and bt % 2 == 1
                                               and not wcorr),
                                         perf_mode=DR)
                        if wcorr:
                            # W-corr: (Wlo pair) @ (xhi pair)
                            nc.tensor.matmul(
                                po, w1a_tiles[e][:, 2 * t2:2 * t2 + 2, 1, :],
                                xt[:, :, 1, bs], start=False,
                                stop=(stop and bt % 2 == 1),
                                perf_mode=DR)

                    def _silu_h1(e, ph1_e):
                        h1_tiles[e] = stream.tile([H, Bs], F32R, tag="h1",
                                                  bufs=4, name=f"h1_{e}")
                        for i in range(2):
                            bs = bass.ts(i, 512)
                            nc.scalar.activation(h1_tiles[e][:, bs],
                                                 ph1_e[i][:], Silu,
                                                 bias=b1_sb[:, e:e + 1],
                                                 scale=S_L1)

                    for gi, grp in enumerate(GROUPS):
                        ph1 = {}
                        for e in grp:
                            # group 1's first expert rides the gate banks
                            # (idle after softmax): the acc rotation keeps
                            # two spare slots through group 0's L2 block
                            tg, nb = ("gate", 2) if (gi >= 1 and e == grp[0]) \
                                else ("acc", 6)
                            ph1[e] = [psA.tile([128, 512], F32, tag=tg,
                                               bufs=nb, name=f"ph1_{e}_{i}")
                                      for i in range(2)]

                        # delayed experts start late (their W1 DMA is
                        # staggered so xc keeps streaming) and catch up on
                        # the last K-tile pairs after the loop; expert e
                        # processes pairs in arrival order 0,1,2,...
                        if gi == 0:
                            delay = {e: DELAYS[i] if i < len(DELAYS) else
                                     2 * i + 1 for i, e in enumerate(grp)}
                        else:
                            delay = {e: 0 for e in grp}

                        nxt = GROUPS[gi + 1] if gi + 1 < len(GROUPS) else []
                        prefetch_at = {}
                        if gi == 0:
                            # own group staggered in halves, next group's
                            # loads late in the step loop
                            for i, e in enumerate(grp[1:]):
                                s0 = 2 * i + 1
                                prefetch_at.setdefault(s0, []).extend(
                                    [(e, 0), (e, 1)])
                            for i, e in enumerate(nxt):
                                prefetch_at.setdefault(
                                    NT2 - len(nxt) + i, []).append((e, None))
                        else:
                            for i, e in enumerate(nxt):
                                prefetch_at[1 + i] = [(e, None)]

                        for step in range(NT2):
                            t2 = step
                            if gi == 0 and _rep == 0:
                                if XC_HALVED:
                                    # b-halves: bt0/1 DRs can start after
                                    # half the slab has landed
                                    nc.sync.dma_start(
                                        xt_tiles[t2][:, :, :, :512],
                                        xc[t2][:, :, :, :512])
                                    nc.sync.dma_start(
                                        xt_tiles[t2][:, :, :, 512:],
                                        xc[t2][:, :, :, 512:])
                                else:
                                    nc.sync.dma_start(xt_tiles[t2][:], xc[t2])
                            if gi == 0 and step == 0:
                                # issued after xt0 so x wins the DMA race
                                _w1_load(grp[0], halves=True)
                            for pe_, hv in prefetch_at.get(step, []):
                                if hv is None:
                                    _w1_load(pe_)
                                else:
                                    _w1_load_half(pe_, hv)
                            if gi == 0 and step == 4:
                                _load_smalls()
                            if gi == 0:
                                for bt in range(NBT):
                                    po = pgate[bt // 2][:, (bt % 2) * BT:
                                                        (bt % 2 + 1) * BT]
                                    xt = xt_tiles[t2]
                                    bs = bass.ts(bt, BT)
                                    nc.tensor.matmul(
                                        po, gw_sb[:, 2 * t2:2 * t2 + 2, 0, :],
                                        xt[:, :, 1, bs],
                                        start=(t2 == 0 and bt % 2 == 0),
                                        stop=False, perf_mode=DR)
                                    nc.tensor.matmul(
                                        po, gw_sb[:, 2 * t2, :, :],
                                        xt[:, 0, :, bs], start=False,
                                        stop=False, perf_mode=DR)
                                    nc.tensor.matmul(
                                        po, gw_sb[:, 2 * t2 + 1, :, :],
                                        xt[:, 1, :, bs], start=False,
                                        stop=(t2 == NT2 - 1 and bt % 2 == 1),
                                        perf_mode=DR)
                            for e in grp:
                                if step < delay[e]:
                                    continue
                                ct2 = step - delay[e]
                                for bt in range(NBT):
                                    _l1_drs(e, ph1[e], ct2, bt,
                                            start=(step == delay[e]),
                                            stop=(step == NT2 - 1
                                                  and delay[e] == 0))

                        if gi == 0:
                            # softmax stage 1 issued before the catch-up so
                            # the ACT exp latency hides under catch-up DRs
                            expT, arT, recip = {}, {}, {}
                            for i in range(2):
                                expT[i] = stream.tile([E, 512], F32R,
                                                      tag="expT", bufs=2,
                                                      name=f"expT{i}")
                                nc.scalar.activation(expT[i][:],
                                                     pgate[i][0:E, :],
                                                     Exp, bias=gb_sb[:],
                                                     scale=S_L1)
                            # catch-up K-tile pairs skipped while W1 was in
                            # flight (same accumulation groups); each
                            # expert's silu issues as soon as it closes
                            for e in grp:
                                d = delay[e]
                                for j, ct2 in enumerate(range(NT2 - d, NT2)):
                                    for bt in range(NBT):
                                        _l1_drs(e, ph1[e], ct2, bt,
                                                start=False, stop=(j == d - 1))
                                _silu_h1(e, ph1[e])
                                if dbg and e == grp[0]:
                                    nc.sync.dma_start(dbg["h1"],
                                                      h1_tiles[e][:])
                            # softmax: gn8[e, b] = 8 * exp(z/512+gb) / sum_e
                            # expert-sum on the (idle) Pool engine, then DVE
                            # reciprocal * 8 and the normalize multiply
                            for i in range(2):
                                arT[i] = stream.tile([E, 512], F32R,
                                                     tag="arT", bufs=2,
                                                     name=f"arT{i}")
                                nc.gpsimd.partition_all_reduce(
                                    arT[i][:], expT[i][:], E,
                                    bass_isa.ReduceOp.add)
                            for i in range(2):
                                recip[i] = stream.tile([E, 512], F32R,
                                                       tag="recip", bufs=4,
                                                       name=f"recip{i}")
                                r8 = stream.tile([E, 512], F32R,
                                                 tag="recip", bufs=4,
                                                 name=f"r8_{i}")
                                with nc.allow_low_precision(
                                        reason="f32r softmax denom"):
                                    nc.vector.reciprocal(recip[i][:],
                                                         arT[i][:])
                                    nc.vector.tensor_scalar_mul(
                                        r8[:], recip[i][:], S_G)
                                nc.vector.tensor_mul(gn_sb[:, bass.ts(i, 512)],
                                                     expT[i][:], r8[:])
                            if dbg:
                                nc.sync.dma_start(dbg["expT"], expT[0][:])
                                nc.sync.dma_start(dbg["gn"], gn_sb[:])
                        else:
                            for e in grp:
                                _silu_h1(e, ph1[e])

                        if gi == len(GROUPS) - 1:
                            # W3 streams in 4-dc slabs on the sync queue so
                            # the issue never serializes behind the scalar
                            # engine's activation backlog
                            w3_slabs = []
                            for sj in range(3):
                                w3s = stream.tile([128, 4, E, 2, H], F8,
                                                  tag="w3q", bufs=3,
                                                  name=f"w3s{sj}")
                                nc.sync.dma_start(w3s[:], w3c[sj])
                                w3_slabs.append(w3s)

                        # per-expert broadcast of this group's gn8 rows
                        # across 128 partitions on Pool (replaces the
                        # one-hot PE matmuls); Pool is otherwise idle
                        for e in grp:
                            for i in range(2):
                                pgb_tiles[e, i] = stream.tile(
                                    [128, 512], F32R, tag="pgb", bufs=6,
                                    name=f"pgb_{e % 3}_{i}")
                                nc.gpsimd.partition_broadcast(
                                    pgb_tiles[e, i][:],
                                    gn_sb[e:e + 1, bass.ts(i, 512)])

                        # L2 + gating + h2 quantization, batched by stage
                        ph2 = {}
                        for e in grp:
                            for i in range(2):
                                bs = bass.ts(i, 512)
                                ph2[e, i] = psA.tile([H, 512], F32, tag="acc",
                                                     bufs=6,
                                                     name=f"ph2_{e}_{i}")
                                nc.tensor.matmul(ph2[e, i][:], w2_sb[:, e, :],
                                                 h1_tiles[e][:, bs],
                                                 start=True, stop=True)
                        for e in grp:
                            for i in range(2):
                                h2t[e, i] = stream.tile([H, 512], F32,
                                                        tag="h2t", bufs=6,
                                                        name=f"h2t_{e}_{i}")
                                nc.scalar.activation(h2t[e, i][:],
                                                     ph2[e, i][:], Silu,
                                                     bias=b2_sb[:, e:e + 1],
                                                     scale=1.0)
                        for e in grp:
                            hc = h2c_tiles[e // 2]
                            ei = e % 2
                            for i in range(2):
                                bs = bass.ts(i, 512)
                                h2s8 = stream.tile([H, 512], F32, tag="h2s8",
                                                   bufs=4, name=f"h2s8_{e}_{i}")
                                with nc.allow_low_precision(
                                        reason="fp8 hi/lo split"):
                                    nc.vector.tensor_mul(h2s8[:], h2t[e, i][:],
                                                         pgb_tiles[e, i][:])
                                    nc.scalar.activation(hc[:, ei, 1, bs],
                                                         h2s8[:], Copy)
                                    nc.vector.tensor_sub(hc[:, ei, 0, bs],
                                                         h2s8[:],
                                                         hc[:, ei, 1, bs])

                    if dbg:
                        nc.sync.dma_start(dbg["h2c0"], h2c_tiles[0][:])
                    # ---- L3: outT[dc] = sum_e W3[e,dc].T @ h2s[e] ----
                    # Experts 0-5 are emitted before 6-7 in each tile so the
                    # PE can start L3 while the last group's h2 quantization
                    # chain (experts 6/7) is still draining on ACT/DVE.
                    # po tiles come from the same "acc" rotation as phase A
                    # (one shared psum pool -> no pool-close barrier).
                    for dc in range(DCH):
                        if dc == 4:
                            w3s = stream.tile([128, 4, E, 2, H], F8,
                                              tag="w3q", bufs=3, name="w3s3")
                            nc.sync.dma_start(w3s[:], w3c[3])
                            w3_slabs.append(w3s)
                        w3t = w3_slabs[dc // 4]
                        dm = dc % 4
                        for half in range(2):
                            # dc0 rides the gate banks (idle since softmax)
                            # so L3 psum never waits on the last group's
                            # gating chain through the acc rotation
                            if dc == 0:
                                po = psA.tile([128, 512], F32, tag="gate",
                                              bufs=2, name=f"pog{half}")
                            else:
                                po = psA.tile([128, 512], F32, tag="acc",
                                              bufs=6,
                                              name=f"po{(dc * 2 + half) % 6}")
                            for sub in range(2):
                                bt = half * 2 + sub
                                bs = bass.ts(bt, BT)
                                pslice = po[:, sub * BT:(sub + 1) * BT]
                                for j in range(3):
                                    nc.tensor.matmul(
                                        pslice,
                                        w3t[:, dm, 2 * j:2 * j + 2, 0, :],
                                        h2c_tiles[j][:, :, 1, bs],
                                        start=(j == 0 and sub == 0),
                                        stop=False, perf_mode=DR)
                                for e in range(6):
                                    nc.tensor.matmul(
                                        pslice, w3t[:, dm, e, :, :],
                                        h2c_tiles[e // 2][:, e % 2, :, bs],
                                        start=False, stop=False,
                                        perf_mode=DR)
                            for sub in range(2):
                                bt = half * 2 + sub
                                bs = bass.ts(bt, BT)
                                pslice = po[:, sub * BT:(sub + 1) * BT]
                                nc.tensor.matmul(
                                    pslice, w3t[:, dm, 6:8, 0, :],
                                    h2c_tiles[3][:, :, 1, bs],
                                    start=False, stop=False, perf_mode=DR)
                                for e in (6, 7):
                                    nc.tensor.matmul(
                                        pslice, w3t[:, dm, e, :, :],
                                        h2c_tiles[3][:, e % 2, :, bs],
                                        start=False,
                                        stop=(e == 7 and sub == 1),
                                        perf_mode=DR)
                            # copy into the per-dc output staging tile;
                            # one DMA per dc (HWDGE descriptor-gen is the
                            # scarce resource, not bandwidth)
                            with nc.allow_low_precision(
                                    reason="bf16 output"):
                                if dc == DCH - 1:
                                    # tail: one engine per half (a split
                                    # within one tile serializes on the
                                    # tile's write tracking), DMA fired
                                    # immediately per half
                                    o_tl = stream.tile([128, 512], BF16,
                                                       tag="osbt", bufs=2,
                                                       name=f"ot{half}")
                                    if half == 0:
                                        nc.vector.tensor_scalar_mul(
                                            o_tl[:], po[:], S_L3)
                                    else:
                                        nc.scalar.activation(
                                            o_tl[:], po[:], Copy, scale=S_L3)
                                    nc.sync.dma_start(
                                        outT[dc * 128:(dc + 1) * 128,
                                             bass.ts(half, 512)], o_tl[:])
                                    continue
                                if half == 0:
                                    o_sb = stream.tile([128, Bs], BF16,
                                                       tag="osb", bufs=3,
                                                       name=f"osb{dc % 3}")
                                hs = bass.ts(half, 512)
                                if half == 0:
                                    nc.scalar.activation(o_sb[:, hs], po[:],
                                                         Copy, scale=S_L3)
                                else:
                                    nc.vector.tensor_scalar_mul(
                                        o_sb[:, hs], po[:], S_L3)
                            if half == 1:
                                nc.sync.dma_start(
                                    outT[dc * 128:(dc + 1) * 128, :],
                                    o_sb[:])
                                if dbg and dc == 0:
                                    nc.sync.dma_start(dbg["y0"], o_sb[:])

    nc.compile()
    return nc


_MODULE_CACHE = {}


def _get_module(reps=1):
    if reps not in _MODULE_CACHE:
        _MODULE_CACHE[reps] = _build_module(reps)
    return _MODULE_CACHE[reps]


def _hilo(a, scale):
    """Return (hi, lo) e4m3 arrays for a*scale, lo at the SAME scale."""
    s = np.float32(scale)
    hi = (a * s).astype(NPF8)
    lo = (a * s - hi.astype(np.float32)).astype(NPF8)
    return hi, lo


def _prep_shared(gate_w, gate_b, W1, b1, W2, b2, W3):
    # gate weights: [128, DCH, 2, E], pairs (hi, lo)
    gwr = np.zeros((128, DCH, 2 * E), dtype=np.float32)
    gwr[:, :, :E] = gate_w.reshape(DCH, 128, E).transpose(1, 0, 2)
    ghi, glo = _hilo(gwr, S_W)
    gwc = np.ascontiguousarray(np.stack([ghi, glo], axis=2))

    # W1: region A = [E, 128, 2*KW1, 2, H] (hi,lo pairs for K-tile pairs
    # 0..KW1-1), region B = [E, 128, DCH-2*KW1, H] hi only
    w1r = W1.reshape(E, DCH, 128, H).transpose(0, 2, 1, 3)
    w1hi, w1lo = _hilo(w1r, S_W)
    w1a = np.ascontiguousarray(
        np.stack([w1hi[:, :, :2 * KW1], w1lo[:, :, :2 * KW1]], axis=3))
    w1b = np.ascontiguousarray(w1hi[:, :, 2 * KW1:])

    # W3: [DCH, 128, E, 2, H]  (partition is the h contraction dim;
    # W3[e] is [H, D]: lhsT per dc = [128(h), 128(d)])
    w3r = W3.reshape(E, H, DCH, 128).transpose(2, 1, 0, 3)
    # w3r: [DCH, H(128 partitions), E, 128(d cols)]
    w3hi, w3lo = _hilo(w3r, S_W)
    w3c = np.stack([w3hi, w3lo], axis=3)          # [DCH, 128, E, 2, 128]
    w3c = np.ascontiguousarray(
        w3c.reshape(4, 4, 128, E, 2, 128).transpose(0, 2, 1, 3, 4, 5))

    return {
        "gwc": gwc, "gb": np.ascontiguousarray(gate_b.reshape(E, 1)),
        "w1a": w1a, "w1b": w1b, "b1t": np.ascontiguousarray(b1.T),
        "w2": np.ascontiguousarray(W2), "b2t": np.ascontiguousarray(b2.T),
        "w3c": w3c,
    }


def _prep_xc(x_slice):
    """x slice [Bs, D] -> [NT2, 128, 2, 2, Bs] fp8, pairs (lo, hi)."""
    xT = x_slice.T.reshape(DCH, 128, Bs)
    xhi, xlo = _hilo(xT, S_X)
    # [DCH, 128, 2(lo,hi), Bs] -> [NT2, 2, 128, 2, Bs] -> [NT2, 128, 2, 2, Bs]
    st = np.stack([xlo, xhi], axis=2).reshape(NT2, 2, 128, 2, Bs)
    return np.ascontiguousarray(st.transpose(0, 2, 1, 3, 4))


def kernel(x, gate_w, gate_b, W1, b1, W2, b2, W3, b3):
    x = np.asarray(x, dtype=np.float32)
    gate_w = np.asarray(gate_w, dtype=np.float32)
    gate_b = np.asarray(gate_b, dtype=np.float32)
    W1 = np.asarray(W1, dtype=np.float32)
    b1 = np.asarray(b1, dtype=np.float32)
    W2 = np.asarray(W2, dtype=np.float32)
    b2 = np.asarray(b2, dtype=np.float32)
    W3 = np.asarray(W3, dtype=np.float32)
    b3 = np.asarray(b3, dtype=np.float32)

    nc = _get_module(1)
    shared = _prep_shared(gate_w, gate_b, W1, b1, W2, b2, W3)
    in_maps = [{"xc": _prep_xc(x[i * Bs:(i + 1) * Bs, :]), **shared}
               for i in range(NCORES)]
    try:
        res = bass_utils.run_bass_kernel_spmd(
            nc, in_maps, core_ids=list(range(NCORES)))
    except Exception:
        # the axon-tunneled devices occasionally report a transient
        # NRT_EXEC_UNIT_UNRECOVERABLE; one retry after a pause clears it
        import time as _time
        _time.sleep(30)
        res = bass_utils.run_bass_kernel_spmd(
            nc, in_maps, core_ids=list(range(NCORES)))

    out = np.empty((B, D), dtype=np.float32)
    for i in range(NCORES):
        out[i * Bs:(i + 1) * Bs, :] = res.results[i]["outT"].T.astype(
            np.float32)

    if np.any(b3):
        # b3 contributes sum_e gates[b,e] * b3[e,d]; the device kernel skips
        # it (it is zero for this problem's inputs), so patch on host.
        logits = x @ gate_w + gate_b
        m = logits.max(axis=1, keepdims=True)
        p = np.exp(logits - m)
        gates = p / p.sum(axis=1, keepdims=True)
        out += gates @ b3
    return out


# revision 18
# speedup vs baseline: 1.1147x; 1.0082x over previous
"""MoE with adaptive gate on 8 trn2 NeuronCores — fp8 DoubleRow, partial W-corr.

Data-parallel over batch (B/8 = 1024 rows per core), feature-major on chip.
The two big GEMMs (L1: x@W1, L3: h2s@W3) run as float8e4 DoubleRow matmuls
(0.5 cycles/row, two 128-row K-tiles per instruction).

Every fp8 operand is split hi+lo at the SAME scale (lo = e4m3(v - hi)); a
K-tile product (Whi+Wlo)@(xhi+xlo) needs 3 of the 4 cross terms (lo@lo is
~delta^2, dropped).  This version additionally drops the W-correction term
(Wlo@xhi) on L1 K-tile pairs 2..7, keeping it only on pairs 0..1: measured
end-to-end rel err 1.65e-2 vs the 2e-2 gate (numpy-emulated, emulator matches
device to 4 digits).  Per K-tile-pair DR count: pairs 0-1: 3, pairs 2-7: 2.

  main DR   t2: (Whi[2t], xhi[2t]) + (Whi[2t+1], xhi[2t+1])
  x-corr DR t2: (Whi[2t], xlo[2t]) + (Whi[2t+1], xlo[2t+1])
  W-corr DR t2 (t2<2): (Wlo[2t], xhi[2t]) + (Wlo[2t+1], xhi[2t+1])

The gate keeps all 3 terms (its logit error amplifies through softmax), and
L3 keeps all 3 terms per expert.

Softmax helpers run off the PE: the expert-sum uses a gpsimd (Pool engine)
partition_all_reduce, the per-expert gate row broadcast to 128 partitions
uses gpsimd partition_broadcast (PE one-hot matmuls removed).

Per-core pipeline (all matmul scales: x*16, W*32 => psum = 512*z):
  gate:  1.5 DR/K-tile on (gw hi/lo, x hi/lo), softmax via exp(z/512+gb),
         Pool allreduce, DVE reciprocal * 8, DVE mul -> gn8 = 8*gates
  L1:    fp8 DR as above -> silu(psum/512 + b1) -> h1 (f32r)
  L2:    fp32r matmul (K=128 only, DR gains nothing) -> silu -> h2
  gating:h2s8 = h2 * pgb (Pool bcast of gn8 row) via DVE mul; hi=e4m3(h2s8)
         (ACT copy), lo = h2s8 - hi (DVE sub, fp8 out)
  L3:    fp8 DR over (e,h) K-tiles, experts paired -> out = psum/256 -> bf16

Output written transposed [D, Bs] in bf16; host transposes/upcasts back.
"""

import sys

sys.path.insert(0, "/opt/trn_rl_repo")

import numpy as np
import ml_dtypes

import concourse.bass as bass
import concourse.tile as tile
from concourse import bacc, mybir
from concourse import bass_utils
from concourse import bass_isa

B, D, E, H = 8192, 2048, 8, 128
NCORES = 8
Bs = B // NCORES          # batch rows per core
BT = 256                  # DoubleRow moving tile (rhs free = 2*BT = 512)
NBT = Bs // BT            # 4 b-tiles per core
DCH = D // 128            # 16 K-tiles over D
NT2 = DCH // 2            # 8 K-tile pairs
KW1 = 2                   # K-tile pairs with the L1 W-corr term kept
GROUPS = [[0, 1, 2], [3, 4, 5], [6, 7]]

F32 = mybir.dt.float32
F32R = mybir.dt.float32r
F8 = mybir.dt.float8e4
BF16 = mybir.dt.bfloat16
NPF8 = ml_dtypes.float8_e4m3
NPBF16 = ml_dtypes.bfloat16
Silu = mybir.ActivationFunctionType.Silu
Exp = mybir.ActivationFunctionType.Exp
Copy = mybir.ActivationFunctionType.Copy
DR = mybir.MatmulPerfMode.DoubleRow

import os
DELAYS = tuple(int(v) for v in os.environ.get("K_DELAYS", "0,2,3").split(","))
XC_HALVED = int(os.environ.get("K_XCHALF", "0"))

S_X = 16.0                # x stored as x*16 in fp8
S_W = 32.0                # weights stored as W*32 in fp8
S_L1 = 1.0 / (S_X * S_W)  # psum of L1/gate = 512 * true value
S_G = 8.0                 # gates folded with x8 for h2s quantization
S_L3 = 1.0 / (S_G * S_W)  # L3 psum = 256 * true value


def _build_module(reps=1):
    nc = bacc.Bacc("TRN2", target_bir_lowering=False, debug=False,
                   num_devices=NCORES)

    # activations interleave (lo, hi) on axis 2; weights (hi, lo) on axis 3/2
    xc = nc.dram_tensor("xc", [NT2, 128, 2, 2, Bs], F8,
                        kind="ExternalInput").ap()
    # gate stationary padded to 16 columns: dual-fp8 LdWeights requires the
    # stationary free width >= 16 (ISA s3_lw_dual_fp8_restrictions)
    gwc = nc.dram_tensor("gwc", [128, DCH, 2, 2 * E], F8,
                         kind="ExternalInput").ap()
    gb = nc.dram_tensor("gb", [E, 1], F32, kind="ExternalInput").ap()
    # W1 in two contiguous regions: A = K-tile pairs 0..KW1-1 with hi+lo
    # planes (W-corr kept there), B = remaining tiles hi plane only
    w1a = nc.dram_tensor("w1a", [E, 128, 2 * KW1, 2, H], F8,
                         kind="ExternalInput").ap()
    w1b = nc.dram_tensor("w1b", [E, 128, DCH - 2 * KW1, H], F8,
                         kind="ExternalInput").ap()
    b1t = nc.dram_tensor("b1t", [H, E], F32, kind="ExternalInput").ap()
    w2 = nc.dram_tensor("w2", [E, H, H], F32R, kind="ExternalInput").ap()
    b2t = nc.dram_tensor("b2t", [H, E], F32, kind="ExternalInput").ap()
    # W3 in 4-dc slabs, partition-major within a slab so one slab DMA is
    # a straight copy into the [128, 4, E, 2, H] SBUF tile
    w3c = nc.dram_tensor("w3c", [DCH // 4, 128, 4, E, 2, H], F8,
                         kind="ExternalInput").ap()
    outT = nc.dram_tensor("outT", [D, Bs], BF16, kind="ExternalOutput").ap()
    dbg = {}
    if os.environ.get("K_DEBUG"):
        dbg["expT"] = nc.dram_tensor("d_expT", [E, 512], F32R,
                                     kind="ExternalOutput").ap()
        dbg["gn"] = nc.dram_tensor("d_gn", [E, Bs], F32R,
                                   kind="ExternalOutput").ap()
        dbg["h1"] = nc.dram_tensor("d_h1", [H, Bs], F32R,
                                   kind="ExternalOutput").ap()
        dbg["h2c0"] = nc.dram_tensor("d_h2c0", [128, 2, 2, Bs], F8,
                                     kind="ExternalOutput").ap()
        dbg["y0"] = nc.dram_tensor("d_y0", [128, Bs], BF16,
                                   kind="ExternalOutput").ap()

    with tile.TileContext(nc) as tc:
        with (
            tc.tile_pool(name="persist", bufs=1) as persist,
            tc.tile_pool(name="stream", bufs=2) as stream,
        ):
            # gate weights first on the sync queue: the first PE work
            # (gate DRs on xc slab 0) needs only gwc + xt0
            gw_sb = persist.tile([128, DCH, 2, 2 * E], F8, tag="gw")
            nc.sync.dma_start(gw_sb[:], gwc[:])
            gb_sb = persist.tile([E, 1], F32, tag="gb")
            b1_sb = persist.tile([H, E], F32, tag="b1")
            b2_sb = persist.tile([H, E], F32, tag="b2")
            w2_sb = persist.tile([H, E, H], F32R, tag="w2")

            def _load_smalls():
                # tiny transfers on the scalar queue: they cost HWDGE slots
                # but ~zero DMA-engine time; w2 goes on the sync queue after
                # xt7 (queue order = transfer priority)
                nc.scalar.dma_start(gb_sb[:], gb[:])
                nc.scalar.dma_start(b1_sb[:], b1t[:])
                nc.scalar.dma_start(b2_sb[:], b2t[:])

            xt_tiles = [persist.tile([128, 2, 2, Bs], F8, tag="xT", bufs=NT2,
                                     name=f"xt{t2}") for t2 in range(NT2)]
            gn_sb = persist.tile([E, Bs], F32R, tag="gn")
            # h2c pair tiles: [h, expert-in-pair, (lo,hi), b]
            h2c_tiles = [persist.tile([128, 2, 2, Bs], F8, tag="h2c", bufs=4,
                                      name=f"h2c{j}") for j in range(4)]

            for _rep in range(reps):
                with tc.tile_pool(name="psumA", bufs=1, space="PSUM") as psA:
                    # 2 gate psum tiles (16 partitions: 8 real experts +
                    # 8 zero pad), 2 softmax groups per tile
                    pgate = [psA.tile([2 * E, 512], F32, tag="gate", bufs=2,
                                      name=f"pg{i}") for i in range(2)]
                    w1a_tiles = {}
                    w1b_tiles = {}
                    h1_tiles = {}
                    h2t = {}
                    pgb_tiles = {}

                    def _w1_load_half(e, hv):
                        # half 0: K-tile pairs 0..KW1-1 both planes (hi+lo,
                        # W-corr kept there); half 1: remaining tiles hi only.
                        # All W1 goes on the sync queue: its FIFO order vs
                        # the xc slabs is the DMA priority schedule.
                        if hv == 0:
                            w1a_tiles[e] = stream.tile(
                                [128, 2 * KW1, 2, H], F8, tag="w1a", bufs=5,
                                name=f"w1a_{e}")
                            nc.sync.dma_start(w1a_tiles[e][:], w1a[e])
                        else:
                            w1b_tiles[e] = stream.tile(
                                [128, DCH - 2 * KW1, H], F8, tag="w1b",
                                bufs=5, name=f"w1b_{e}")
                            nc.sync.dma_start(w1b_tiles[e][:], w1b[e])

                    def _w1_load(e, halves=False):
                        _w1_load_half(e, 0)
                        _w1_load_half(e, 1)

                    def _l1_drs(e, ph, t2, bt, start, stop):
                        """2-3 DRs for K-tile pair t2 into psum slice for bt.

                        One psum bank (2KB zero region) holds two bt slices:
                        start only on the bank's first matmul (bt even),
                        stop only on its last (bt odd).
                        """
                        po = ph[bt // 2][:, (bt % 2) * BT:(bt % 2 + 1) * BT]
                        xt = xt_tiles[t2]
                        bs = bass.ts(bt, BT)
                        wcorr = t2 < KW1
                        if wcorr:
                            whi = w1a_tiles[e][:, 2 * t2:2 * t2 + 2, 0, :]
                        else:
                            t0 = 2 * (t2 - KW1)
                            whi = w1b_tiles[e][:, t0:t0 + 2, :]
                        # main: (Whi pair) @ (xhi pair)
                        nc.tensor.matmul(po, whi, xt[:, :, 1, bs],
                                         start=(start and bt % 2 == 0),
                                         stop=False, perf_mode=DR)
                        # x-corr: (Whi pair) @ (xlo pair)
                        nc.tensor.matmul(po, whi, xt[:, :, 0, bs],
                                         start=False,
                                         stop=(stop and bt % 2 == 1
                                               and not wcorr),
                                         perf_mode=DR)
                        if wcorr:
                            # W-corr: (Wlo pair) @ (xhi pair)
                            nc.tensor.matmul(
                                po, w1a_tiles[e][:, 2 * t2:2 * t2 + 2, 1, :],
                                xt[:, :, 1, bs], start=False,
                                stop=(stop and bt % 2 == 1),
                                perf_mode=DR)

                    def _silu_h1(e, ph1_e):
                        h1_tiles[e] = stream.tile([H, Bs], F32R, tag="h1",
                                                  bufs=4, name=f"h1_{e}")
                        for i in range(2):
                            bs = bass.ts(i, 512)
                            nc.scalar.activation(h1_tiles[e][:, bs],
                                                 ph1_e[i][:], Silu,
                                                 bias=b1_sb[:, e:e + 1],
                                                 scale=S_L1)

                    for gi, grp in enumerate(GROUPS):
                        ph1 = {}
                        for e in grp:
                            # group 1's first expert rides the gate banks
                            # (idle after softmax): the acc rotation keeps
                            # two spare slots through group 0's L2 block
                            tg, nb = ("gate", 2) if (gi >= 1 and e == grp[0]) \
                                else ("acc", 6)
                            ph1[e] = [psA.tile([128, 512], F32, tag=tg,
                                               bufs=nb, name=f"ph1_{e}_{i}")
                                      for i in range(2)]

                        # delayed experts start late (their W1 DMA is
                        # staggered so xc keeps streaming) and catch up on
                        # the last K-tile pairs after the loop; expert e
                        # processes pairs in arrival order 0,1,2,...
                        if gi == 0:
                            delay = {e: DELAYS[i] if i < len(DELAYS) else
                                     2 * i + 1 for i, e in enumerate(grp)}
                        else:
                            delay = {e: 0 for e in grp}

                        nxt = GROUPS[gi + 1] if gi + 1 < len(GROUPS) else []
                        prefetch_at = {}
                        if gi == 0:
                            # own group staggered one half per step so each
                            # xc slab's transfer is delayed by at most one
                            # small W1 piece; the next group's loads (and
                            # w2) queue after xt7 -> they transfer only
                            # once the x stream is done
                            prefetch_at = {
                                0: [(grp[0], 1)],
                                1: [(grp[1], 0)],
                                2: [(grp[1], 1)],
                                3: [(grp[2], 0)],
                                4: [(grp[2], 1)],
                                NT2 - 1: [(nxt[0], None), ("w2", None),
                                          (nxt[1], None), (nxt[2], None)],
                            }
                            # first expert's hi/lo region + smalls issued
                            # before xt0 (region A is small; smalls are
                            # HWDGE-only noise on the scalar queue)
                            _w1_load_half(grp[0], 0)
                            _load_smalls()
                        else:
                            for i, e in enumerate(nxt):
                                prefetch_at[1 + i] = [(e, None)]

                        for step in range(NT2):
                            t2 = step
                            if gi == 0 and _rep == 0:
                                if XC_HALVED:
                                    # b-halves: bt0/1 DRs can start after
                                    # half the slab has landed
                                    nc.sync.dma_start(
                                        xt_tiles[t2][:, :, :, :512],
                                        xc[t2][:, :, :, :512])
                                    nc.sync.dma_start(
                                        xt_tiles[t2][:, :, :, 512:],
                                        xc[t2][:, :, :, 512:])
                                else:
                                    nc.sync.dma_start(xt_tiles[t2][:], xc[t2])
                            for pe_, hv in prefetch_at.get(step, []):
                                if pe_ == "w2":
                                    nc.sync.dma_start(
                                        w2_sb[:],
                                        w2.rearrange("e h k -> h e k"))
                                elif hv is None:
                                    _w1_load(pe_)
                                else:
                                    _w1_load_half(pe_, hv)
                            if gi == 0:
                                for bt in range(NBT):
                                    po = pgate[bt // 2][:, (bt % 2) * BT:
                                                        (bt % 2 + 1) * BT]
                                    xt = xt_tiles[t2]
                                    bs = bass.ts(bt, BT)
                                    nc.tensor.matmul(
                                        po, gw_sb[:, 2 * t2:2 * t2 + 2, 0, :],
                                        xt[:, :, 1, bs],
                                        start=(t2 == 0 and bt % 2 == 0),
                                        stop=False, perf_mode=DR)
                                    nc.tensor.matmul(
                                        po, gw_sb[:, 2 * t2, :, :],
                                        xt[:, 0, :, bs], start=False,
                                        stop=False, perf_mode=DR)
                                    nc.tensor.matmul(
                                        po, gw_sb[:, 2 * t2 + 1, :, :],
                                        xt[:, 1, :, bs], start=False,
                                        stop=(t2 == NT2 - 1 and bt % 2 == 1),
                                        perf_mode=DR)
                            for e in grp:
                                if step < delay[e]:
                                    continue
                                ct2 = step - delay[e]
                                for bt in range(NBT):
                                    _l1_drs(e, ph1[e], ct2, bt,
                                            start=(step == delay[e]),
                                            stop=(step == NT2 - 1
                                                  and delay[e] == 0))

                        if gi == 0:
                            # softmax stage 1 issued before the catch-up so
                            # the ACT exp latency hides under catch-up DRs
                            expT, arT, recip = {}, {}, {}
                            for i in range(2):
                                expT[i] = stream.tile([E, 512], F32R,
                                                      tag="expT", bufs=2,
                                                      name=f"expT{i}")
                                nc.scalar.activation(expT[i][:],
                                                     pgate[i][0:E, :],
                                                     Exp, bias=gb_sb[:],
                                                     scale=S_L1)
                            # catch-up K-tile pairs skipped while W1 was in
                            # flight (same accumulation groups); each
                            # expert's silu issues as soon as it closes
                            for e in grp:
                                d = delay[e]
                                for j, ct2 in enumerate(range(NT2 - d, NT2)):
                                    for bt in range(NBT):
                                        _l1_drs(e, ph1[e], ct2, bt,
                                                start=False, stop=(j == d - 1))
                                _silu_h1(e, ph1[e])
                                if dbg and e == grp[0]:
                                    nc.sync.dma_start(dbg["h1"],
                                                      h1_tiles[e][:])
                            # softmax: gn8[e, b] = 8 * exp(z/512+gb) / sum_e
                            # expert-sum on the (idle) Pool engine, then DVE
                            # reciprocal * 8 and the normalize multiply
                            for i in range(2):
                                arT[i] = stream.tile([E, 512], F32R,
                                                     tag="arT", bufs=2,
                                                     name=f"arT{i}")
                                nc.gpsimd.partition_all_reduce(
                                    arT[i][:], expT[i][:], E,
                                    bass_isa.ReduceOp.add)
                            for i in range(2):
                                recip[i] = stream.tile([E, 512], F32R,
                                                       tag="recip", bufs=4,
                                                       name=f"recip{i}")
                                r8 = stream.tile([E, 512], F32R,
                                                 tag="recip", bufs=4,
                                                 name=f"r8_{i}")
                                with nc.allow_low_precision(
                                        reason="f32r softmax denom"):
                                    nc.vector.reciprocal(recip[i][:],
                                                         arT[i][:])
                                    nc.vector.tensor_scalar_mul(
                                        r8[:], recip[i][:], S_G)
                                nc.vector.tensor_mul(gn_sb[:, bass.ts(i, 512)],
                                                     expT[i][:], r8[:])
                            if dbg:
                                nc.sync.dma_start(dbg["expT"], expT[0][:])
                                nc.sync.dma_start(dbg["gn"], gn_sb[:])
                        else:
                            for e in grp:
                                _silu_h1(e, ph1[e])

                        if gi == len(GROUPS) - 1:
                            # W3 streams in 4-dc slabs on the sync queue so
                            # the issue never serializes behind the scalar
                            # engine's activation backlog
                            w3_slabs = []
                            for sj in range(3):
                                w3s = stream.tile([128, 4, E, 2, H], F8,
                                                  tag="w3q", bufs=3,
                                                  name=f"w3s{sj}")
                                nc.sync.dma_start(w3s[:], w3c[sj])
                                w3_slabs.append(w3s)

                        # per-expert broadcast of this group's gn8 rows
                        # across 128 partitions on Pool (replaces the
                        # one-hot PE matmuls); Pool is otherwise idle
                        for e in grp:
                            for i in range(2):
                                pgb_tiles[e, i] = stream.tile(
                                    [128, 512], F32R, tag="pgb", bufs=6,
                                    name=f"pgb_{e % 3}_{i}")
                                nc.gpsimd.partition_broadcast(
                                    pgb_tiles[e, i][:],
                                    gn_sb[e:e + 1, bass.ts(i, 512)])

                        # L2 + gating + h2 quantization, batched by stage.
                        # i-major order: the b-half-0 chain for every expert
                        # completes first, so L3's late DRs (which consume
                        # h2c half 0 before half 1) start sooner.
                        ph2 = {}
                        for i in range(2):
                            for e in grp:
                                bs = bass.ts(i, 512)
                                ph2[e, i] = psA.tile([H, 512], F32, tag="acc",
                                                     bufs=6,
                                                     name=f"ph2_{e}_{i}")
                                nc.tensor.matmul(ph2[e, i][:], w2_sb[:, e, :],
                                                 h1_tiles[e][:, bs],
                                                 start=True, stop=True)
                        for i in range(2):
                            for e in grp:
                                h2t[e, i] = stream.tile([H, 512], F32,
                                                        tag="h2t", bufs=6,
                                                        name=f"h2t_{e}_{i}")
                                nc.scalar.activation(h2t[e, i][:],
                                                     ph2[e, i][:], Silu,
                                                     bias=b2_sb[:, e:e + 1],
                                                     scale=1.0)
                        for i in range(2):
                            for e in grp:
                                hc = h2c_tiles[e // 2]
                                ei = e % 2
                                bs = bass.ts(i, 512)
                                h2s8 = stream.tile([H, 512], F32, tag="h2s8",
                                                   bufs=4, name=f"h2s8_{e}_{i}")
                                with nc.allow_low_precision(
                                        reason="fp8 hi/lo split"):
                                    nc.vector.tensor_mul(h2s8[:], h2t[e, i][:],
                                                         pgb_tiles[e, i][:])
                                    nc.scalar.activation(hc[:, ei, 1, bs],
                                                         h2s8[:], Copy)
                                    nc.vector.tensor_sub(hc[:, ei, 0, bs],
                                                         h2s8[:],
                                                         hc[:, ei, 1, bs])

                    if dbg:
                        nc.sync.dma_start(dbg["h2c0"], h2c_tiles[0][:])
                    # ---- L3: outT[dc] = sum_e W3[e,dc].T @ h2s[e] ----
                    # Experts 0-5 are emitted before 6-7 in each tile so the
                    # PE can start L3 while the last group's h2 quantization
                    # chain (experts 6/7) is still draining on ACT/DVE.
                    # po tiles come from the same "acc" rotation as phase A
                    # (one shared psum pool -> no pool-close barrier).
                    # The first 4 dc open all 8 psum banks and emit their
                    # expert-0-5 DRs as one batch (~5us of PE work) before
                    # the first expert-6/7 DR, hiding the last group's
                    # gating-chain latency.
                    def _l3_early(po, w3t, dm, half):
                        for sub in range(2):
                            bt = half * 2 + sub
                            bs = bass.ts(bt, BT)
                            pslice = po[:, sub * BT:(sub + 1) * BT]
                            for j in range(3):
                                nc.tensor.matmul(
                                    pslice,
                                    w3t[:, dm, 2 * j:2 * j + 2, 0, :],
                                    h2c_tiles[j][:, :, 1, bs],
                                    start=(j == 0 and sub == 0),
                                    stop=False, perf_mode=DR)
                            for e in range(6):
                                nc.tensor.matmul(
                                    pslice, w3t[:, dm, e, :, :],
                                    h2c_tiles[e // 2][:, e % 2, :, bs],
                                    start=False, stop=False,
                                    perf_mode=DR)

                    def _l3_late(po, w3t, dm, half):
                        for sub in range(2):
                            bt = half * 2 + sub
                            bs = bass.ts(bt, BT)
                            pslice = po[:, sub * BT:(sub + 1) * BT]
                            nc.tensor.matmul(
                                pslice, w3t[:, dm, 6:8, 0, :],
                                h2c_tiles[3][:, :, 1, bs],
                                start=False, stop=False, perf_mode=DR)
                            for e in (6, 7):
                                nc.tensor.matmul(
                                    pslice, w3t[:, dm, e, :, :],
                                    h2c_tiles[3][:, e % 2, :, bs],
                                    start=False,
                                    stop=(e == 7 and sub == 1),
                                    perf_mode=DR)

                    po_head = {}
                    for dc in range(4):
                        for half in range(2):
                            if dc == 0:
                                po = psA.tile([128, 512], F32, tag="gate",
                                              bufs=2, name=f"pog{half}")
                            else:
                                po = psA.tile([128, 512], F32, tag="acc",
                                              bufs=6,
                                              name=f"po{(dc * 2 + half) % 6}")
                            po_head[dc, half] = po
                            _l3_early(po, w3_slabs[0], dc, half)

                    for dc in range(DCH):
                        if dc == 4:
                            w3s = stream.tile([128, 4, E, 2, H], F8,
                                              tag="w3q", bufs=3, name="w3s3")
                            nc.sync.dma_start(w3s[:], w3c[3])
                            w3_slabs.append(w3s)
                        w3t = w3_slabs[dc // 4]
                        dm = dc % 4
                        for half in range(2):
                            # dc0 rides the gate banks (idle since softmax)
                            # so L3 psum never waits on the last group's
                            # gating chain through the acc rotation
                            if dc < 4:
                                po = po_head[dc, half]
                            else:
                                po = psA.tile([128, 512], F32, tag="acc",
                                              bufs=6,
                                              name=f"po{(dc * 2 + half) % 6}")
                                _l3_early(po, w3t, dm, half)
                            _l3_late(po, w3t, dm, half)
                            # copy into the per-dc output staging tile;
                            # one DMA per dc (HWDGE descriptor-gen is the
                            # scarce resource, not bandwidth)
                            with nc.allow_low_precision(
                                    reason="bf16 output"):
                                if dc == DCH - 1:
                                    # tail: one engine per half (a split
                                    # within one tile serializes on the
                                    # tile's write tracking), DMA fired
                                    # immediately per half
                                    o_tl = stream.tile([128, 512], BF16,
                                                       tag="osbt", bufs=2,
                                                       name=f"ot{half}")
                                    if half == 0:
                                        nc.vector.tensor_scalar_mul(
                                            o_tl[:], po[:], S_L3)
                                    else:
                                        nc.scalar.activation(
                                            o_tl[:], po[:], Copy, scale=S_L3)
                                    nc.sync.dma_start(
                                        outT[dc * 128:(dc + 1) * 128,
                                             bass.ts(half, 512)], o_tl[:])
                                    continue
                                if half == 0:
                                    o_sb = stream.tile([128, Bs], BF16,
                                                       tag="osb", bufs=3,
                                                       name=f"osb{dc % 3}")
                                hs = bass.ts(half, 512)
                                if half == 0:
                                    nc.scalar.activation(o_sb[:, hs], po[:],
                                                         Copy, scale=S_L3)
                                else:
                                    nc.vector.tensor_scalar_mul(
                                        o_sb[:, hs], po[:], S_L3)
                            if half == 1:
                                nc.sync.dma_start(
                                    outT[dc * 128:(dc + 1) * 128, :],
                                    o_sb[:])
                                if dbg and dc == 0:
                                    nc.sync.dma_start(dbg["y0"], o_sb[:])

    nc.compile()
    return nc


_MODULE_CACHE = {}


def _get_module(reps=1):
    if reps not in _MODULE_CACHE:
        _MODULE_CACHE[reps] = _build_module(reps)
    return _MODULE_CACHE[reps]


def _hilo(a, scale):
    """Return (hi, lo) e4m3 arrays for a*scale, lo at the SAME scale."""
    s = np.float32(scale)
    hi = (a * s).astype(NPF8)
    lo = (a * s - hi.astype(np.float32)).astype(NPF8)
    return hi, lo


def _prep_shared(gate_w, gate_b, W1, b1, W2, b2, W3):
    # gate weights: [128, DCH, 2, E], pairs (hi, lo)
    gwr = np.zeros((128, DCH, 2 * E), dtype=np.float32)
    gwr[:, :, :E] = gate_w.reshape(DCH, 128, E).transpose(1, 0, 2)
    ghi, glo = _hilo(gwr, S_W)
    gwc = np.ascontiguousarray(np.stack([ghi, glo], axis=2))

    # W1: region A = [E, 128, 2*KW1, 2, H] (hi,lo pairs for K-tile pairs
    # 0..KW1-1), region B = [E, 128, DCH-2*KW1, H] hi only
    w1r = W1.reshape(E, DCH, 128, H).transpose(0, 2, 1, 3)
    w1hi, w1lo = _hilo(w1r, S_W)
    w1a = np.ascontiguousarray(
        np.stack([w1hi[:, :, :2 * KW1], w1lo[:, :, :2 * KW1]], axis=3))
    w1b = np.ascontiguousarray(w1hi[:, :, 2 * KW1:])

    # W3: [DCH, 128, E, 2, H]  (partition is the h contraction dim;
    # W3[e] is [H, D]: lhsT per dc = [128(h), 128(d)])
    w3r = W3.reshape(E, H, DCH, 128).transpose(2, 1, 0, 3)
    # w3r: [DCH, H(128 partitions), E, 128(d cols)]
    w3hi, w3lo = _hilo(w3r, S_W)
    w3c = np.stack([w3hi, w3lo], axis=3)          # [DCH, 128, E, 2, 128]
    w3c = np.ascontiguousarray(
        w3c.reshape(4, 4, 128, E, 2, 128).transpose(0, 2, 1, 3, 4, 5))

    return {
        "gwc": gwc, "gb": np.ascontiguousarray(gate_b.reshape(E, 1)),
        "w1a": w1a, "w1b": w1b, "b1t": np.ascontiguousarray(b1.T),
        "w2": np.ascontiguousarray(W2), "b2t": np.ascontiguousarray(b2.T),
        "w3c": w3c,
    }


def _prep_xc(x_slice):
    """x slice [Bs, D] -> [NT2, 128, 2, 2, Bs] fp8, pairs (lo, hi)."""
    xT = x_slice.T.reshape(DCH, 128, Bs)
    xhi, xlo = _hilo(xT, S_X)
    # [DCH, 128, 2(lo,hi), Bs] -> [NT2, 2, 128, 2, Bs] -> [NT2, 128, 2, 2, Bs]
    st = np.stack([xlo, xhi], axis=2).reshape(NT2, 2, 128, 2, Bs)
    return np.ascontiguousarray(st.transpose(0, 2, 1, 3, 4))


def kernel(x, gate_w, gate_b, W1, b1, W2, b2, W3, b3):
    x = np.asarray(x, dtype=np.float32)
    gate_w = np.asarray(gate_w, dtype=np.float32)
    gate_b = np.asarray(gate_b, dtype=np.float32)
    W1 = np.asarray(W1, dtype=np.float32)
    b1 = np.asarray(b1, dtype=np.float32)
    W2 = np.asarray(W2, dtype=np.float32)
    b2 = np.asarray(b2, dtype=np.float32)
    W3 = np.asarray(W3, dtype=np.float32)
    b3 = np.asarray(b3, dtype=np.float32)

    nc = _get_module(1)
    shared = _prep_shared(gate_w, gate_b, W1, b1, W2, b2, W3)
    in_maps = [{"xc": _prep_xc(x[i * Bs:(i + 1) * Bs, :]), **shared}
               for i in range(NCORES)]
    try:
        res = bass_utils.run_bass_kernel_spmd(
            nc, in_maps, core_ids=list(range(NCORES)))
    except Exception:
        # the axon-tunneled devices occasionally report a transient
        # NRT_EXEC_UNIT_UNRECOVERABLE; one retry after a pause clears it
        import time as _time
        _time.sleep(30)
        res = bass_utils.run_bass_kernel_spmd(
            nc, in_maps, core_ids=list(range(NCORES)))

    out = np.empty((B, D), dtype=np.float32)
    for i in range(NCORES):
        out[i * Bs:(i + 1) * Bs, :] = res.results[i]["outT"].T.astype(
            np.float32)

    if np.any(b3):
        # b3 contributes sum_e gates[b,e] * b3[e,d]; the device kernel skips
        # it (it is zero for this problem's inputs), so patch on host.
        logits = x @ gate_w + gate_b
        m = logits.max(axis=1, keepdims=True)
        p = np.exp(logits - m)
        gates = p / p.sum(axis=1, keepdims=True)
        out += gates @ b3
    return out


# revision 23
# speedup vs baseline: 1.1639x; 1.0441x over previous
"""MoE with adaptive gate on 8 trn2 NeuronCores — fp8 DoubleRow, partial W-corr.

Data-parallel over batch (B/8 = 1024 rows per core), feature-major on chip.
The two big GEMMs (L1: x@W1, L3: h2s@W3) run as float8e4 DoubleRow matmuls
(0.5 cycles/row, two 128-row K-tiles per instruction).

Every fp8 operand is split hi+lo at the SAME scale (lo = e4m3(v - hi)); a
K-tile product (Whi+Wlo)@(xhi+xlo) needs 3 of the 4 cross terms (lo@lo is
~delta^2, dropped).  This version additionally drops the W-correction term
(Wlo@xhi) on L1 K-tile pairs 2..7, keeping it only on pairs 0..1: measured
end-to-end rel err 1.65e-2 vs the 2e-2 gate (numpy-emulated, emulator matches
device to 4 digits).  Per K-tile-pair DR count: pairs 0-1: 3, pairs 2-7: 2.

  main DR   t2: (Whi[2t], xhi[2t]) + (Whi[2t+1], xhi[2t+1])
  x-corr DR t2: (Whi[2t], xlo[2t]) + (Whi[2t+1], xlo[2t+1])
  W-corr DR t2 (t2<2): (Wlo[2t], xhi[2t]) + (Wlo[2t+1], xhi[2t+1])

The gate keeps all 3 terms (its logit error amplifies through softmax), and
L3 keeps all 3 terms per expert.

Softmax helpers run off the PE: the expert-sum uses a gpsimd (Pool engine)
partition_all_reduce, the per-expert gate row broadcast to 128 partitions
uses gpsimd partition_broadcast (PE one-hot matmuls removed).

Per-core pipeline (all matmul scales: x*16, W*32 => psum = 512*z):
  gate:  1.5 DR/K-tile on (gw hi/lo, x hi/lo), softmax via exp(z/512+gb),
         Pool allreduce, DVE reciprocal * 8, DVE mul -> gn8 = 8*gates
  L1:    fp8 DR as above -> silu(psum/512 + b1) -> h1 (f32r)
  L2:    fp32r matmul (K=128 only, DR gains nothing) -> silu -> h2
  gating:h2s8 = h2 * pgb (Pool bcast of gn8 row) via DVE mul; hi=e4m3(h2s8)
         (ACT copy), lo = h2s8 - hi (DVE sub, fp8 out)
  L3:    fp8 DR over (e,h) K-tiles, experts paired -> out = psum/256 -> bf16

Output written transposed [D, Bs] in bf16; host transposes/upcasts back.
"""

import sys

sys.path.insert(0, "/opt/trn_rl_repo")

import numpy as np
import ml_dtypes

import concourse.bass as bass
import concourse.tile as tile
from concourse import bacc, mybir
from concourse import bass_utils
from concourse import bass_isa

B, D, E, H = 8192, 2048, 8, 128
NCORES = 8
Bs = B // NCORES          # batch rows per core
BT = 256                  # DoubleRow moving tile (rhs free = 2*BT = 512)
NBT = Bs // BT            # 4 b-tiles per core
DCH = D // 128            # 16 K-tiles over D
NT2 = DCH // 2            # 8 K-tile pairs
KW1 = 2                   # K-tile pairs with the L1 W-corr term kept
GROUPS = [[0, 1, 2], [3, 4, 5], [6, 7]]

F32 = mybir.dt.float32
F32R = mybir.dt.float32r
F8 = mybir.dt.float8e4
BF16 = mybir.dt.bfloat16
NPF8 = ml_dtypes.float8_e4m3
NPBF16 = ml_dtypes.bfloat16
Silu = mybir.ActivationFunctionType.Silu
Exp = mybir.ActivationFunctionType.Exp
Copy = mybir.ActivationFunctionType.Copy
DR = mybir.MatmulPerfMode.DoubleRow

import os
DELAYS = tuple(int(v) for v in os.environ.get("K_DELAYS", "0,2,4").split(","))
XC_HALVED = int(os.environ.get("K_XCHALF", "0"))

S_X = 16.0                # x stored as x*16 in fp8
S_W = 32.0                # weights stored as W*32 in fp8
S_L1 = 1.0 / (S_X * S_W)  # psum of L1/gate = 512 * true value
S_G = 8.0                 # gates folded with x8 for h2s quantization
S_L3 = 1.0 / (S_G * S_W)  # L3 psum = 256 * true value


def _build_module(reps=1):
    nc = bacc.Bacc("TRN2", target_bir_lowering=False, debug=False,
                   num_devices=NCORES)

    # activations interleave (lo, hi) on axis 2; weights (hi, lo) on axis 3/2
    xc = nc.dram_tensor("xc", [NT2, 128, 2, 2, Bs], F8,
                        kind="ExternalInput").ap()
    # gate stationary padded to 16 columns: dual-fp8 LdWeights requires the
    # stationary free width >= 16 (ISA s3_lw_dual_fp8_restrictions)
    gwc = nc.dram_tensor("gwc", [128, DCH, 2, 2 * E], F8,
                         kind="ExternalInput").ap()
    gb = nc.dram_tensor("gb", [E, 1], F32, kind="ExternalInput").ap()
    # W1 in two contiguous regions: A = K-tile pairs 0..KW1-1 with hi+lo
    # planes (W-corr kept there), B = remaining tiles hi plane only
    w1a = nc.dram_tensor("w1a", [E, 128, 2 * KW1, 2, H], F8,
                         kind="ExternalInput").ap()
    w1b = nc.dram_tensor("w1b", [E, 128, DCH - 2 * KW1, H], F8,
                         kind="ExternalInput").ap()
    b1t = nc.dram_tensor("b1t", [H, E], F32, kind="ExternalInput").ap()
    w2 = nc.dram_tensor("w2", [E, H, H], F32R, kind="ExternalInput").ap()
    b2t = nc.dram_tensor("b2t", [H, E], F32, kind="ExternalInput").ap()
    # W3 in 4-dc slabs, partition-major within a slab so one slab DMA is
    # a straight copy into the [128, 4, E, 2, H] SBUF tile
    w3c = nc.dram_tensor("w3c", [DCH // 4, 128, 4, E, 2, H], F8,
                         kind="ExternalInput").ap()
    outT = nc.dram_tensor("outT", [D, Bs], BF16, kind="ExternalOutput").ap()
    dbg = {}
    if os.environ.get("K_DEBUG"):
        dbg["expT"] = nc.dram_tensor("d_expT", [E, 512], F32R,
                                     kind="ExternalOutput").ap()
        dbg["gn"] = nc.dram_tensor("d_gn", [E, Bs], F32R,
                                   kind="ExternalOutput").ap()
        dbg["h1"] = nc.dram_tensor("d_h1", [H, Bs], F32R,
                                   kind="ExternalOutput").ap()
        dbg["h2c0"] = nc.dram_tensor("d_h2c0", [128, 2, 2, Bs], F8,
                                     kind="ExternalOutput").ap()
        dbg["y0"] = nc.dram_tensor("d_y0", [128, Bs], BF16,
                                   kind="ExternalOutput").ap()

    with tile.TileContext(nc) as tc:
        with (
            tc.tile_pool(name="persist", bufs=1) as persist,
            tc.tile_pool(name="stream", bufs=2) as stream,
        ):
            # gate weights first on the sync queue: the first PE work
            # (gate DRs on xc slab 0) needs only gwc + xt0
            gw_sb = persist.tile([128, DCH, 2, 2 * E], F8, tag="gw")
            nc.sync.dma_start(gw_sb[:], gwc[:])
            gb_sb = persist.tile([E, 1], F32, tag="gb")
            b1_sb = persist.tile([H, E], F32, tag="b1")
            b2_sb = persist.tile([H, E], F32, tag="b2")
            w2_sb = persist.tile([H, E, H], F32R, tag="w2")

            def _load_smalls():
                # on the sync queue after xt7: their HWDGE slots must not
                # delay the x stream (queue order = transfer priority)
                nc.sync.dma_start(gb_sb[:], gb[:])
                nc.sync.dma_start(b1_sb[:], b1t[:])
                nc.sync.dma_start(b2_sb[:], b2t[:])

            xt_tiles = [persist.tile([128, 2, 2, Bs], F8, tag="xT", bufs=NT2,
                                     name=f"xt{t2}") for t2 in range(NT2)]
            gn_sb = persist.tile([E, Bs], F32R, tag="gn")
            # h2c pair tiles: [h, expert-in-pair, (lo,hi), b]
            h2c_tiles = [persist.tile([128, 2, 2, Bs], F8, tag="h2c", bufs=4,
                                      name=f"h2c{j}") for j in range(4)]

            for _rep in range(reps):
                with tc.tile_pool(name="psumA", bufs=1, space="PSUM") as psA:
                    # 2 gate psum tiles (16 partitions: 8 real experts +
                    # 8 zero pad), 2 softmax groups per tile
                    pgate = [psA.tile([2 * E, 512], F32, tag="gate", bufs=2,
                                      name=f"pg{i}") for i in range(2)]
                    w1a_tiles = {}
                    w1b_tiles = {}
                    h1_tiles = {}
                    h2t = {}
                    pgb_tiles = {}

                    def _w1_load_half(e, hv):
                        # half 0: K-tile pairs 0..KW1-1 both planes (hi+lo,
                        # W-corr kept there); half 1: remaining tiles hi only.
                        # All W1 goes on the sync queue: its FIFO order vs
                        # the xc slabs is the DMA priority schedule.
                        if hv == 0:
                            w1a_tiles[e] = stream.tile(
                                [128, 2 * KW1, 2, H], F8, tag="w1a", bufs=5,
                                name=f"w1a_{e}")
                            nc.sync.dma_start(w1a_tiles[e][:], w1a[e])
                        else:
                            w1b_tiles[e] = stream.tile(
                                [128, DCH - 2 * KW1, H], F8, tag="w1b",
                                bufs=5, name=f"w1b_{e}")
                            nc.sync.dma_start(w1b_tiles[e][:], w1b[e])

                    def _w1_load(e, halves=False):
                        _w1_load_half(e, 0)
                        _w1_load_half(e, 1)

                    def _l1_drs(e, ph, t2, bt, start, stop):
                        """2-3 DRs for K-tile pair t2 into psum slice for bt.

                        One psum bank (2KB zero region) holds two bt slices:
                        start only on the bank's first matmul (bt even),
                        stop only on its last (bt odd).
                        """
                        po = ph[bt // 2][:, (bt % 2) * BT:(bt % 2 + 1) * BT]
                        xt = xt_tiles[t2]
                        bs = bass.ts(bt, BT)
                        wcorr = t2 < KW1
                        if wcorr:
                            whi = w1a_tiles[e][:, 2 * t2:2 * t2 + 2, 0, :]
                        else:
                            t0 = 2 * (t2 - KW1)
                            whi = w1b_tiles[e][:, t0:t0 + 2, :]
                        # main: (Whi pair) @ (xhi pair)
                        nc.tensor.matmul(po, whi, xt[:, :, 1, bs],
                                         start=(start and bt % 2 == 0),
                                         stop=False, perf_mode=DR)
                        # x-corr: (Whi pair) @ (xlo pair)
                        nc.tensor.matmul(po, whi, xt[:, :, 0, bs],
                                         start=False,
                                         stop=(stop and bt % 2 == 1
                                               and not wcorr),
                                         perf_mode=DR)
                        if wcorr:
                            # W-corr: (Wlo pair) @ (xhi pair)
                            nc.tensor.matmul(
                                po, w1a_tiles[e][:, 2 * t2:2 * t2 + 2, 1, :],
                                xt[:, :, 1, bs], start=False,
                                stop=(stop and bt % 2 == 1),
                                perf_mode=DR)

                    def _silu_h1(e, ph1_e):
                        h1_tiles[e] = stream.tile([H, Bs], F32R, tag="h1",
                                                  bufs=4, name=f"h1_{e}")
                        for i in range(2):
                            bs = bass.ts(i, 512)
                            nc.scalar.activation(h1_tiles[e][:, bs],
                                                 ph1_e[i][:], Silu,
                                                 bias=b1_sb[:, e:e + 1],
                                                 scale=S_L1)

                    for gi, grp in enumerate(GROUPS):
                        ph1 = {}
                        for e in grp:
                            # group 1's first expert rides the gate banks
                            # (idle after softmax): the acc rotation keeps
                            # two spare slots through group 0's L2 block
                            tg, nb = ("gate", 2) if (gi >= 1 and e == grp[0]) \
                                else ("acc", 6)
                            ph1[e] = [psA.tile([128, 512], F32, tag=tg,
                                               bufs=nb, name=f"ph1_{e}_{i}")
                                      for i in range(2)]

                        # delayed experts start late (their W1 DMA is
                        # staggered so xc keeps streaming) and catch up on
                        # the last K-tile pairs after the loop; expert e
                        # processes pairs in arrival order 0,1,2,...
                        if gi == 0:
                            delay = {e: DELAYS[i] if i < len(DELAYS) else
                                     2 * i + 1 for i, e in enumerate(grp)}
                        else:
                            delay = {e: 0 for e in grp}

                        nxt = GROUPS[gi + 1] if gi + 1 < len(GROUPS) else []
                        prefetch_at = {}
                        if gi == 0:
                            # one small W1 piece between consecutive xc
                            # slabs; the next group's loads, smalls, and w2
                            # queue after xt7 -> they transfer only once the
                            # x stream is done (single-queue FIFO order)
                            prefetch_at = {
                                0: [(grp[0], 0)],
                                1: [(grp[0], 1)],
                                2: [(grp[1], 0)],
                                3: [(grp[1], 1)],
                                4: [(grp[2], 0)],
                                5: [(grp[2], 1)],
                                NT2 - 1: [("smalls", None),
                                          (nxt[0], None), ("w2", None),
                                          (nxt[1], None), (nxt[2], None)],
                            }
                        else:
                            for i, e in enumerate(nxt):
                                prefetch_at[1 + i] = [(e, None)]

                        if gi == len(GROUPS) - 1:
                            # last group, all data already on-chip: run in
                            # two b-half batches so the i=0 psums close
                            # (and the h1->L2->gating chain starts) while
                            # the second batch's DRs still feed the PE
                            for ih in range(2):
                                for t2 in range(NT2):
                                    for e in grp:
                                        for bt in (2 * ih, 2 * ih + 1):
                                            _l1_drs(e, ph1[e], t2, bt,
                                                    start=(t2 == 0),
                                                    stop=(t2 == NT2 - 1))
                                for e in grp:
                                    if ih == 0:
                                        h1_tiles[e] = stream.tile(
                                            [H, Bs], F32R, tag="h1",
                                            bufs=4, name=f"h1_{e}")
                                    nc.scalar.activation(
                                        h1_tiles[e][:, bass.ts(ih, 512)],
                                        ph1[e][ih][:], Silu,
                                        bias=b1_sb[:, e:e + 1], scale=S_L1)
                            steps = []
                        else:
                            steps = range(NT2)
                        for step in steps:
                            t2 = step
                            if gi == 0 and _rep == 0:
                                if step == 0:
                                    # first slab in b-halves: the gate DRs
                                    # for bt 0/1 start ~0.7us sooner
                                    nc.sync.dma_start(
                                        xt_tiles[t2][:, :, :, :512],
                                        xc[t2][:, :, :, :512])
                                    nc.sync.dma_start(
                                        xt_tiles[t2][:, :, :, 512:],
                                        xc[t2][:, :, :, 512:])
                                else:
                                    nc.sync.dma_start(xt_tiles[t2][:], xc[t2])
                            for pe_, hv in prefetch_at.get(step, []):
                                if pe_ == "w2":
                                    nc.sync.dma_start(
                                        w2_sb[:],
                                        w2.rearrange("e h k -> h e k"))
                                elif pe_ == "smalls":
                                    _load_smalls()
                                elif hv is None:
                                    _w1_load(pe_)
                                else:
                                    _w1_load_half(pe_, hv)
                            if gi == 0:
                                for bt in range(NBT):
                                    po = pgate[bt // 2][:, (bt % 2) * BT:
                                                        (bt % 2 + 1) * BT]
                                    xt = xt_tiles[t2]
                                    bs = bass.ts(bt, BT)
                                    nc.tensor.matmul(
                                        po, gw_sb[:, 2 * t2:2 * t2 + 2, 0, :],
                                        xt[:, :, 1, bs],
                                        start=(t2 == 0 and bt % 2 == 0),
                                        stop=False, perf_mode=DR)
                                    nc.tensor.matmul(
                                        po, gw_sb[:, 2 * t2, :, :],
                                        xt[:, 0, :, bs], start=False,
                                        stop=False, perf_mode=DR)
                                    nc.tensor.matmul(
                                        po, gw_sb[:, 2 * t2 + 1, :, :],
                                        xt[:, 1, :, bs], start=False,
                                        stop=(t2 == NT2 - 1 and bt % 2 == 1),
                                        perf_mode=DR)
                            for e in grp:
                                if step < delay[e]:
                                    continue
                                ct2 = step - delay[e]
                                for bt in range(NBT):
                                    _l1_drs(e, ph1[e], ct2, bt,
                                            start=(step == delay[e]),
                                            stop=(step == NT2 - 1
                                                  and delay[e] == 0))

                        if gi == 0:
                            # softmax stage 1 issued before the catch-up so
                            # the ACT exp latency hides under catch-up DRs
                            expT, arT, recip = {}, {}, {}
                            for i in range(2):
                                expT[i] = stream.tile([E, 512], F32R,
                                                      tag="expT", bufs=2,
                                                      name=f"expT{i}")
                                nc.scalar.activation(expT[i][:],
                                                     pgate[i][0:E, :],
                                                     Exp, bias=gb_sb[:],
                                                     scale=S_L1)
                            # catch-up K-tile pairs skipped while W1 was in
                            # flight (same accumulation groups); each
                            # expert's silu issues as soon as it closes
                            for e in grp:
                                d = delay[e]
                                for j, ct2 in enumerate(range(NT2 - d, NT2)):
                                    for bt in range(NBT):
                                        _l1_drs(e, ph1[e], ct2, bt,
                                                start=False, stop=(j == d - 1))
                                _silu_h1(e, ph1[e])
                                if dbg and e == grp[0]:
                                    nc.sync.dma_start(dbg["h1"],
                                                      h1_tiles[e][:])
                            # softmax: gn8[e, b] = 8 * exp(z/512+gb) / sum_e
                            # expert-sum on the (idle) Pool engine, then DVE
                            # reciprocal * 8 and the normalize multiply
                            for i in range(2):
                                arT[i] = stream.tile([E, 512], F32R,
                                                     tag="arT", bufs=2,
                                                     name=f"arT{i}")
                                nc.gpsimd.partition_all_reduce(
                                    arT[i][:], expT[i][:], E,
                                    bass_isa.ReduceOp.add)
                            for i in range(2):
                                recip[i] = stream.tile([E, 512], F32R,
                                                       tag="recip", bufs=4,
                                                       name=f"recip{i}")
                                r8 = stream.tile([E, 512], F32R,
                                                 tag="recip", bufs=4,
                                                 name=f"r8_{i}")
                                with nc.allow_low_precision(
                                        reason="f32r softmax denom"):
                                    nc.vector.reciprocal(recip[i][:],
                                                         arT[i][:])
                                    nc.vector.tensor_scalar_mul(
                                        r8[:], recip[i][:], S_G)
                                nc.vector.tensor_mul(gn_sb[:, bass.ts(i, 512)],
                                                     expT[i][:], r8[:])
                            if dbg:
                                nc.sync.dma_start(dbg["expT"], expT[0][:])
                                nc.sync.dma_start(dbg["gn"], gn_sb[:])
                        elif gi < len(GROUPS) - 1:
                            for e in grp:
                                _silu_h1(e, ph1[e])

                        if gi == len(GROUPS) - 1:
                            # W3 streams in 4-dc slabs on the sync queue so
                            # the issue never serializes behind the scalar
                            # engine's activation backlog
                            w3_slabs = []
                            for sj in range(3):
                                w3s = stream.tile([128, 4, E, 2, H], F8,
                                                  tag="w3q", bufs=3,
                                                  name=f"w3s{sj}")
                                nc.sync.dma_start(w3s[:], w3c[sj])
                                w3_slabs.append(w3s)

                        # per-expert broadcast of this group's gn8 rows
                        # across 128 partitions on Pool (replaces the
                        # one-hot PE matmuls); Pool is otherwise idle
                        for e in grp:
                            for i in range(2):
                                pgb_tiles[e, i] = stream.tile(
                                    [128, 512], F32R, tag="pgb", bufs=6,
                                    name=f"pgb_{e % 3}_{i}")
                                nc.gpsimd.partition_broadcast(
                                    pgb_tiles[e, i][:],
                                    gn_sb[e:e + 1, bass.ts(i, 512)])

                        # L2 + gating + h2 quantization, batched by stage.
                        # i-major order: the b-half-0 chain for every expert
                        # completes first, so L3's late DRs (which consume
                        # h2c half 0 before half 1) start sooner.
                        ph2 = {}
                        for i in range(2):
                            for e in grp:
                                bs = bass.ts(i, 512)
                                ph2[e, i] = psA.tile([H, 512], F32, tag="acc",
                                                     bufs=6,
                                                     name=f"ph2_{e}_{i}")
                                nc.tensor.matmul(ph2[e, i][:], w2_sb[:, e, :],
                                                 h1_tiles[e][:, bs],
                                                 start=True, stop=True)
                        for i in range(2):
                            for e in grp:
                                h2t[e, i] = stream.tile([H, 512], F32,
                                                        tag="h2t", bufs=6,
                                                        name=f"h2t_{e}_{i}")
                                nc.scalar.activation(h2t[e, i][:],
                                                     ph2[e, i][:], Silu,
                                                     bias=b2_sb[:, e:e + 1],
                                                     scale=1.0)
                        for i in range(2):
                            for e in grp:
                                hc = h2c_tiles[e // 2]
                                ei = e % 2
                                bs = bass.ts(i, 512)
                                h2s8 = stream.tile([H, 512], F32, tag="h2s8",
                                                   bufs=4, name=f"h2s8_{e}_{i}")
                                with nc.allow_low_precision(
                                        reason="fp8 hi/lo split"):
                                    nc.vector.tensor_mul(h2s8[:], h2t[e, i][:],
                                                         pgb_tiles[e, i][:])
                                    nc.scalar.activation(hc[:, ei, 1, bs],
                                                         h2s8[:], Copy)
                                    nc.vector.tensor_sub(hc[:, ei, 0, bs],
                                                         h2s8[:],
                                                         hc[:, ei, 1, bs])

                    if dbg:
                        nc.sync.dma_start(dbg["h2c0"], h2c_tiles[0][:])
                    # ---- L3: outT[dc] = sum_e W3[e,dc].T @ h2s[e] ----
                    # Experts 0-5 are emitted before 6-7 in each tile so the
                    # PE can start L3 while the last group's h2 quantization
                    # chain (experts 6/7) is still draining on ACT/DVE.
                    # po tiles come from the same "acc" rotation as phase A
                    # (one shared psum pool -> no pool-close barrier).
                    # The first 4 dc open all 8 psum banks and emit their
                    # expert-0-5 DRs as one batch (~5us of PE work) before
                    # the first expert-6/7 DR, hiding the last group's
                    # gating-chain latency.
                    def _l3_early(po, w3t, dm, half):
                        for sub in range(2):
                            bt = half * 2 + sub
                            bs = bass.ts(bt, BT)
                            pslice = po[:, sub * BT:(sub + 1) * BT]
                            for j in range(3):
                                nc.tensor.matmul(
                                    pslice,
                                    w3t[:, dm, 2 * j:2 * j + 2, 0, :],
                                    h2c_tiles[j][:, :, 1, bs],
                                    start=(j == 0 and sub == 0),
                                    stop=False, perf_mode=DR)
                            for e in range(6):
                                nc.tensor.matmul(
                                    pslice, w3t[:, dm, e, :, :],
                                    h2c_tiles[e // 2][:, e % 2, :, bs],
                                    start=False, stop=False,
                                    perf_mode=DR)

                    def _l3_late(po, w3t, dm, half):
                        for sub in range(2):
                            bt = half * 2 + sub
                            bs = bass.ts(bt, BT)
                            pslice = po[:, sub * BT:(sub + 1) * BT]
                            nc.tensor.matmul(
                                pslice, w3t[:, dm, 6:8, 0, :],
                                h2c_tiles[3][:, :, 1, bs],
                                start=False, stop=False, perf_mode=DR)
                            for e in (6, 7):
                                nc.tensor.matmul(
                                    pslice, w3t[:, dm, e, :, :],
                                    h2c_tiles[3][:, e % 2, :, bs],
                                    start=False,
                                    stop=(e == 7 and sub == 1),
                                    perf_mode=DR)

                    po_head = {}
                    for dc in range(4):
                        for half in range(2):
                            if dc == 0:
                                po = psA.tile([128, 512], F32, tag="gate",
                                              bufs=2, name=f"pog{half}")
                            else:
                                po = psA.tile([128, 512], F32, tag="acc",
                                              bufs=6,
                                              name=f"po{(dc * 2 + half) % 6}")
                            po_head[dc, half] = po
                            _l3_early(po, w3_slabs[0], dc, half)

                    for dc in range(DCH):
                        if dc == 4:
                            w3s = stream.tile([128, 4, E, 2, H], F8,
                                              tag="w3q", bufs=3, name="w3s3")
                            nc.sync.dma_start(w3s[:], w3c[3])
                            w3_slabs.append(w3s)
                        w3t = w3_slabs[dc // 4]
                        dm = dc % 4
                        for half in range(2):
                            # dc0 rides the gate banks (idle since softmax)
                            # so L3 psum never waits on the last group's
                            # gating chain through the acc rotation
                            if dc < 4:
                                po = po_head[dc, half]
                            else:
                                po = psA.tile([128, 512], F32, tag="acc",
                                              bufs=6,
                                              name=f"po{(dc * 2 + half) % 6}")
                                _l3_early(po, w3t, dm, half)
                            _l3_late(po, w3t, dm, half)
                            # copy into the per-dc output staging tile;
                            # one DMA per dc (HWDGE descriptor-gen is the
                            # scarce resource, not bandwidth)
                            with nc.allow_low_precision(
                                    reason="bf16 output"):
                                if dc == DCH - 1:
                                    # tail: one engine per half (a split
                                    # within one tile serializes on the
                                    # tile's write tracking), DMA fired
                                    # immediately per half
                                    o_tl = stream.tile([128, 512], BF16,
                                                       tag="osbt", bufs=2,
                                                       name=f"ot{half}")
                                    if half == 0:
                                        nc.vector.tensor_scalar_mul(
                                            o_tl[:], po[:], S_L3)
                                    else:
                                        nc.scalar.activation(
                                            o_tl[:], po[:], Copy, scale=S_L3)
                                    nc.sync.dma_start(
                                        outT[dc * 128:(dc + 1) * 128,
                                             bass.ts(half, 512)], o_tl[:])
                                    continue
                                if half == 0:
                                    o_sb = stream.tile([128, Bs], BF16,
                                                       tag="osb", bufs=3,
                                                       name=f"osb{dc % 3}")
                                hs = bass.ts(half, 512)
                                if half == 0:
                                    nc.scalar.activation(o_sb[:, hs], po[:],
                                                         Copy, scale=S_L3)
                                else:
                                    nc.vector.tensor_scalar_mul(
                                        o_sb[:, hs], po[:], S_L3)
                            if half == 1:
                                nc.sync.dma_start(
                                    outT[dc * 128:(dc + 1) * 128, :],
                                    o_sb[:])
                                if dbg and dc == 0:
                                    nc.sync.dma_start(dbg["y0"], o_sb[:])

    nc.compile()
    return nc


_MODULE_CACHE = {}


def _get_module(reps=1):
    if reps not in _MODULE_CACHE:
        _MODULE_CACHE[reps] = _build_module(reps)
    return _MODULE_CACHE[reps]


def _hilo(a, scale):
    """Return (hi, lo) e4m3 arrays for a*scale, lo at the SAME scale."""
    s = np.float32(scale)
    hi = (a * s).astype(NPF8)
    lo = (a * s - hi.astype(np.float32)).astype(NPF8)
    return hi, lo


def _prep_shared(gate_w, gate_b, W1, b1, W2, b2, W3):
    # gate weights: [128, DCH, 2, E], pairs (hi, lo)
    gwr = np.zeros((128, DCH, 2 * E), dtype=np.float32)
    gwr[:, :, :E] = gate_w.reshape(DCH, 128, E).transpose(1, 0, 2)
    ghi, glo = _hilo(gwr, S_W)
    gwc = np.ascontiguousarray(np.stack([ghi, glo], axis=2))

    # W1: region A = [E, 128, 2*KW1, 2, H] (hi,lo pairs for K-tile pairs
    # 0..KW1-1), region B = [E, 128, DCH-2*KW1, H] hi only
    w1r = W1.reshape(E, DCH, 128, H).transpose(0, 2, 1, 3)
    w1hi, w1lo = _hilo(w1r, S_W)
    w1a = np.ascontiguousarray(
        np.stack([w1hi[:, :, :2 * KW1], w1lo[:, :, :2 * KW1]], axis=3))
    w1b = np.ascontiguousarray(w1hi[:, :, 2 * KW1:])

    # W3: [DCH, 128, E, 2, H]  (partition is the h contraction dim;
    # W3[e] is [H, D]: lhsT per dc = [128(h), 128(d)])
    w3r = W3.reshape(E, H, DCH, 128).transpose(2, 1, 0, 3)
    # w3r: [DCH, H(128 partitions), E, 128(d cols)]
    w3hi, w3lo = _hilo(w3r, S_W)
    w3c = np.stack([w3hi, w3lo], axis=3)          # [DCH, 128, E, 2, 128]
    w3c = np.ascontiguousarray(
        w3c.reshape(4, 4, 128, E, 2, 128).transpose(0, 2, 1, 3, 4, 5))

    return {
        "gwc": gwc, "gb": np.ascontiguousarray(gate_b.reshape(E, 1)),
        "w1a": w1a, "w1b": w1b, "b1t": np.ascontiguousarray(b1.T),
        "w2": np.ascontiguousarray(W2), "b2t": np.ascontiguousarray(b2.T),
        "w3c": w3c,
    }


def _prep_xc(x_slice):
    """x slice [Bs, D] -> [NT2, 128, 2, 2, Bs] fp8, pairs (lo, hi)."""
    xT = x_slice.T.reshape(DCH, 128, Bs)
    xhi, xlo = _hilo(xT, S_X)
    # [DCH, 128, 2(lo,hi), Bs] -> [NT2, 2, 128, 2, Bs] -> [NT2, 128, 2, 2, Bs]
    st = np.stack([xlo, xhi], axis=2).reshape(NT2, 2, 128, 2, Bs)
    return np.ascontiguousarray(st.transpose(0, 2, 1, 3, 4))


def kernel(x, gate_w, gate_b, W1, b1, W2, b2, W3, b3):
    x = np.asarray(x, dtype=np.float32)
    gate_w = np.asarray(gate_w, dtype=np.float32)
    gate_b = np.asarray(gate_b, dtype=np.float32)
    W1 = np.asarray(W1, dtype=np.float32)
    b1 = np.asarray(b1, dtype=np.float32)
    W2 = np.asarray(W2, dtype=np.float32)
    b2 = np.asarray(b2, dtype=np.float32)
    W3 = np.asarray(W3, dtype=np.float32)
    b3 = np.asarray(b3, dtype=np.float32)

    nc = _get_module(1)
    shared = _prep_shared(gate_w, gate_b, W1, b1, W2, b2, W3)
    in_maps = [{"xc": _prep_xc(x[i * Bs:(i + 1) * Bs, :]), **shared}
               for i in range(NCORES)]
    try:
        res = bass_utils.run_bass_kernel_spmd(
            nc, in_maps, core_ids=list(range(NCORES)))
    except Exception:
        # the axon-tunneled devices occasionally report a transient
        # NRT_EXEC_UNIT_UNRECOVERABLE; one retry after a pause clears it
        import time as _time
        _time.sleep(30)
        res = bass_utils.run_bass_kernel_spmd(
            nc, in_maps, core_ids=list(range(NCORES)))

    out = np.empty((B, D), dtype=np.float32)
    for i in range(NCORES):
        out[i * Bs:(i + 1) * Bs, :] = res.results[i]["outT"].T.astype(
            np.float32)

    if np.any(b3):
        # b3 contributes sum_e gates[b,e] * b3[e,d]; the device kernel skips
        # it (it is zero for this problem's inputs), so patch on host.
        logits = x @ gate_w + gate_b
        m = logits.max(axis=1, keepdims=True)
        p = np.exp(logits - m)
        gates = p / p.sum(axis=1, keepdims=True)
        out += gates @ b3
    return out


# revision 24
# speedup vs baseline: 1.1773x; 1.0115x over previous
"""MoE with adaptive gate on 8 trn2 NeuronCores — fp8 DoubleRow, partial W-corr.

Data-parallel over batch (B/8 = 1024 rows per core), feature-major on chip.
The two big GEMMs (L1: x@W1, L3: h2s@W3) run as float8e4 DoubleRow matmuls
(0.5 cycles/row, two 128-row K-tiles per instruction).

Every fp8 operand is split hi+lo at the SAME scale (lo = e4m3(v - hi)); a
K-tile product (Whi+Wlo)@(xhi+xlo) needs 3 of the 4 cross terms (lo@lo is
~delta^2, dropped).  This version additionally drops the W-correction term
(Wlo@xhi) on L1 K-tile pairs 2..7, keeping it only on pairs 0..1: measured
end-to-end rel err 1.65e-2 vs the 2e-2 gate (numpy-emulated, emulator matches
device to 4 digits).  Per K-tile-pair DR count: pairs 0-1: 3, pairs 2-7: 2.

  main DR   t2: (Whi[2t], xhi[2t]) + (Whi[2t+1], xhi[2t+1])
  x-corr DR t2: (Whi[2t], xlo[2t]) + (Whi[2t+1], xlo[2t+1])
  W-corr DR t2 (t2<2): (Wlo[2t], xhi[2t]) + (Wlo[2t+1], xhi[2t+1])

The gate keeps all 3 terms (its logit error amplifies through softmax), and
L3 keeps all 3 terms per expert.

Softmax helpers run off the PE: the expert-sum uses a gpsimd (Pool engine)
partition_all_reduce, the per-expert gate row broadcast to 128 partitions
uses gpsimd partition_broadcast (PE one-hot matmuls removed).

Per-core pipeline (all matmul scales: x*16, W*32 => psum = 512*z):
  gate:  1.5 DR/K-tile on (gw hi/lo, x hi/lo), softmax via exp(z/512+gb),
         Pool allreduce, DVE reciprocal * 8, DVE mul -> gn8 = 8*gates
  L1:    fp8 DR as above -> silu(psum/512 + b1) -> h1 (f32r)
  L2:    fp32r matmul (K=128 only, DR gains nothing) -> silu -> h2
  gating:h2s8 = h2 * pgb (Pool bcast of gn8 row) via DVE mul; hi=e4m3(h2s8)
         (ACT copy), lo = h2s8 - hi (DVE sub, fp8 out)
  L3:    fp8 DR over (e,h) K-tiles, experts paired -> out = psum/256 -> bf16

Output written transposed [D, Bs] in bf16; host transposes/upcasts back.
"""

import sys

sys.path.insert(0, "/opt/trn_rl_repo")

import numpy as np
import ml_dtypes

import concourse.bass as bass
import concourse.tile as tile
from concourse import bacc, mybir
from concourse import bass_utils
from concourse import bass_isa

B, D, E, H = 8192, 2048, 8, 128
NCORES = 8
Bs = B // NCORES          # batch rows per core
BT = 256                  # DoubleRow moving tile (rhs free = 2*BT = 512)
NBT = Bs // BT            # 4 b-tiles per core
DCH = D // 128            # 16 K-tiles over D
NT2 = DCH // 2            # 8 K-tile pairs
KW1 = 2                   # K-tile pairs with the L1 W-corr term kept
GROUPS = [[0, 1, 2], [3, 4, 5], [6, 7]]

F32 = mybir.dt.float32
F32R = mybir.dt.float32r
F8 = mybir.dt.float8e4
BF16 = mybir.dt.bfloat16
NPF8 = ml_dtypes.float8_e4m3
NPBF16 = ml_dtypes.bfloat16
Silu = mybir.ActivationFunctionType.Silu
Exp = mybir.ActivationFunctionType.Exp
Copy = mybir.ActivationFunctionType.Copy
DR = mybir.MatmulPerfMode.DoubleRow

import os
DELAYS = tuple(int(v) for v in os.environ.get("K_DELAYS", "0,2,4").split(","))
XC_HALVED = int(os.environ.get("K_XCHALF", "0"))

S_X = 16.0                # x stored as x*16 in fp8
S_W = 32.0                # weights stored as W*32 in fp8
S_L1 = 1.0 / (S_X * S_W)  # psum of L1/gate = 512 * true value
S_G = 8.0                 # gates folded with x8 for h2s quantization
S_L3 = 1.0 / (S_G * S_W)  # L3 psum = 256 * true value


def _build_module(reps=1):
    nc = bacc.Bacc("TRN2", target_bir_lowering=False, debug=False,
                   num_devices=NCORES)

    # activations interleave (lo, hi) on axis 2; weights (hi, lo) on axis 3/2
    xc = nc.dram_tensor("xc", [NT2, 128, 2, 2, Bs], F8,
                        kind="ExternalInput").ap()
    # gate stationary padded to 16 columns: dual-fp8 LdWeights requires the
    # stationary free width >= 16 (ISA s3_lw_dual_fp8_restrictions)
    gwc = nc.dram_tensor("gwc", [128, DCH, 2, 2 * E], F8,
                         kind="ExternalInput").ap()
    gb = nc.dram_tensor("gb", [E, 1], F32, kind="ExternalInput").ap()
    # W1 in two contiguous regions: A = K-tile pairs 0..KW1-1 with hi+lo
    # planes (W-corr kept there), B = remaining tiles hi plane only
    w1a = nc.dram_tensor("w1a", [E, 128, 2 * KW1, 2, H], F8,
                         kind="ExternalInput").ap()
    w1b = nc.dram_tensor("w1b", [E, 128, DCH - 2 * KW1, H], F8,
                         kind="ExternalInput").ap()
    b1t = nc.dram_tensor("b1t", [H, E], F32, kind="ExternalInput").ap()
    w2 = nc.dram_tensor("w2", [E, H, H], F32R, kind="ExternalInput").ap()
    b2t = nc.dram_tensor("b2t", [H, E], F32, kind="ExternalInput").ap()
    # W3 in 4-dc slabs, partition-major within a slab so one slab DMA is
    # a straight copy into the [128, 4, E, 2, H] SBUF tile
    w3c = nc.dram_tensor("w3c", [DCH // 4, 128, 4, E, 2, H], F8,
                         kind="ExternalInput").ap()
    outT = nc.dram_tensor("outT", [D, Bs], BF16, kind="ExternalOutput").ap()
    dbg = {}
    if os.environ.get("K_DEBUG"):
        dbg["expT"] = nc.dram_tensor("d_expT", [E, 512], F32R,
                                     kind="ExternalOutput").ap()
        dbg["gn"] = nc.dram_tensor("d_gn", [E, Bs], F32R,
                                   kind="ExternalOutput").ap()
        dbg["h1"] = nc.dram_tensor("d_h1", [H, Bs], F32R,
                                   kind="ExternalOutput").ap()
        dbg["h2c0"] = nc.dram_tensor("d_h2c0", [128, 2, 2, Bs], F8,
                                     kind="ExternalOutput").ap()
        dbg["y0"] = nc.dram_tensor("d_y0", [128, Bs], BF16,
                                   kind="ExternalOutput").ap()

    with tile.TileContext(nc) as tc:
        with (
            tc.tile_pool(name="persist", bufs=1) as persist,
            tc.tile_pool(name="stream", bufs=2) as stream,
        ):
            # gate weights first on the sync queue: the first PE work
            # (gate DRs on xc slab 0) needs only gwc + xt0
            gw_sb = persist.tile([128, DCH, 2, 2 * E], F8, tag="gw")
            nc.sync.dma_start(gw_sb[:], gwc[:])
            gb_sb = persist.tile([E, 1], F32, tag="gb")
            b1_sb = persist.tile([H, E], F32, tag="b1")
            b2_sb = persist.tile([H, E], F32, tag="b2")
            w2_sb = persist.tile([H, E, H], F32R, tag="w2")

            def _load_smalls():
                # on the sync queue after xt7: their HWDGE slots must not
                # delay the x stream (queue order = transfer priority)
                nc.sync.dma_start(gb_sb[:], gb[:])
                nc.sync.dma_start(b1_sb[:], b1t[:])
                nc.sync.dma_start(b2_sb[:], b2t[:])

            xt_tiles = [persist.tile([128, 2, 2, Bs], F8, tag="xT", bufs=NT2,
                                     name=f"xt{t2}") for t2 in range(NT2)]
            gn_sb = persist.tile([E, Bs], F32R, tag="gn")
            # h2c pair tiles: [h, expert-in-pair, (lo,hi), b]
            h2c_tiles = [persist.tile([128, 2, 2, Bs], F8, tag="h2c", bufs=4,
                                      name=f"h2c{j}") for j in range(4)]

            for _rep in range(reps):
                with tc.tile_pool(name="psumA", bufs=1, space="PSUM") as psA:
                    # 2 gate psum tiles (16 partitions: 8 real experts +
                    # 8 zero pad), 2 softmax groups per tile
                    pgate = [psA.tile([2 * E, 512], F32, tag="gate", bufs=2,
                                      name=f"pg{i}") for i in range(2)]
                    w1a_tiles = {}
                    w1b_tiles = {}
                    h1_tiles = {}
                    h2t = {}
                    pgb_tiles = {}

                    def _w1_load_half(e, hv):
                        # half 0: K-tile pairs 0..KW1-1 both planes (hi+lo,
                        # W-corr kept there); half 1: remaining tiles hi only.
                        # All W1 goes on the sync queue: its FIFO order vs
                        # the xc slabs is the DMA priority schedule.
                        if hv == 0:
                            w1a_tiles[e] = stream.tile(
                                [128, 2 * KW1, 2, H], F8, tag="w1a", bufs=5,
                                name=f"w1a_{e}")
                            nc.sync.dma_start(w1a_tiles[e][:], w1a[e])
                        else:
                            w1b_tiles[e] = stream.tile(
                                [128, DCH - 2 * KW1, H], F8, tag="w1b",
                                bufs=5, name=f"w1b_{e}")
                            nc.sync.dma_start(w1b_tiles[e][:], w1b[e])

                    def _w1_load(e, halves=False):
                        _w1_load_half(e, 0)
                        _w1_load_half(e, 1)

                    def _l1_drs(e, ph, t2, bt, start, stop):
                        """2-3 DRs for K-tile pair t2 into psum slice for bt.

                        One psum bank (2KB zero region) holds two bt slices:
                        start only on the bank's first matmul (bt even),
                        stop only on its last (bt odd).
                        """
                        po = ph[bt // 2][:, (bt % 2) * BT:(bt % 2 + 1) * BT]
                        xt = xt_tiles[t2]
                        bs = bass.ts(bt, BT)
                        wcorr = t2 < KW1
                        if wcorr:
                            whi = w1a_tiles[e][:, 2 * t2:2 * t2 + 2, 0, :]
                        else:
                            t0 = 2 * (t2 - KW1)
                            whi = w1b_tiles[e][:, t0:t0 + 2, :]
                        # main: (Whi pair) @ (xhi pair)
                        nc.tensor.matmul(po, whi, xt[:, :, 1, bs],
                                         start=(start and bt % 2 == 0),
                                         stop=False, perf_mode=DR)
                        # x-corr: (Whi pair) @ (xlo pair)
                        nc.tensor.matmul(po, whi, xt[:, :, 0, bs],
                                         start=False,
                                         stop=(stop and bt % 2 == 1
                                               and not wcorr),
                                         perf_mode=DR)
                        if wcorr:
                            # W-corr: (Wlo pair) @ (xhi pair)
                            nc.tensor.matmul(
                                po, w1a_tiles[e][:, 2 * t2:2 * t2 + 2, 1, :],
                                xt[:, :, 1, bs], start=False,
                                stop=(stop and bt % 2 == 1),
                                perf_mode=DR)

                    def _silu_h1(e, ph1_e):
                        h1_tiles[e] = stream.tile([H, Bs], F32R, tag="h1",
                                                  bufs=4, name=f"h1_{e}")
                        for i in range(2):
                            bs = bass.ts(i, 512)
                            nc.scalar.activation(h1_tiles[e][:, bs],
                                                 ph1_e[i][:], Silu,
                                                 bias=b1_sb[:, e:e + 1],
                                                 scale=S_L1)

                    for gi, grp in enumerate(GROUPS):
                        ph1 = {}
                        for e in grp:
                            # group 1's first expert rides the gate banks
                            # (idle after softmax): the acc rotation keeps
                            # two spare slots through group 0's L2 block
                            tg, nb = ("gate", 2) if (gi >= 1 and e == grp[0]) \
                                else ("acc", 6)
                            ph1[e] = [psA.tile([128, 512], F32, tag=tg,
                                               bufs=nb, name=f"ph1_{e}_{i}")
                                      for i in range(2)]

                        # delayed experts start late (their W1 DMA is
                        # staggered so xc keeps streaming) and catch up on
                        # the last K-tile pairs after the loop; expert e
                        # processes pairs in arrival order 0,1,2,...
                        if gi == 0:
                            delay = {e: DELAYS[i] if i < len(DELAYS) else
                                     2 * i + 1 for i, e in enumerate(grp)}
                        else:
                            delay = {e: 0 for e in grp}

                        nxt = GROUPS[gi + 1] if gi + 1 < len(GROUPS) else []
                        prefetch_at = {}
                        if gi == 0:
                            # one small W1 piece between consecutive xc
                            # slabs; the next group's loads, smalls, and w2
                            # queue after xt7 -> they transfer only once the
                            # x stream is done (single-queue FIFO order)
                            prefetch_at = {}
                            for i, e in enumerate(grp):
                                d = delay[e]
                                prefetch_at.setdefault(
                                    max(0, d - 1), []).append((e, 0))
                                prefetch_at.setdefault(
                                    min(d + 1, NT2 - 2), []).append((e, 1))
                            prefetch_at.setdefault(NT2 - 1, []).extend(
                                [("smalls", None),
                                 (nxt[0], None), ("w2", None),
                                 (nxt[1], None), (nxt[2], None)])
                        else:
                            for i, e in enumerate(nxt):
                                prefetch_at[1 + i] = [(e, None)]

                        if gi == len(GROUPS) - 1:
                            # last group, all data already on-chip: run in
                            # two b-half batches so the i=0 psums close
                            # (and the h1->L2->gating chain starts) while
                            # the second batch's DRs still feed the PE
                            for ih in range(2):
                                for t2 in range(NT2):
                                    for e in grp:
                                        for bt in (2 * ih, 2 * ih + 1):
                                            _l1_drs(e, ph1[e], t2, bt,
                                                    start=(t2 == 0),
                                                    stop=(t2 == NT2 - 1))
                                for e in grp:
                                    if ih == 0:
                                        h1_tiles[e] = stream.tile(
                                            [H, Bs], F32R, tag="h1",
                                            bufs=4, name=f"h1_{e}")
                                    nc.scalar.activation(
                                        h1_tiles[e][:, bass.ts(ih, 512)],
                                        ph1[e][ih][:], Silu,
                                        bias=b1_sb[:, e:e + 1], scale=S_L1)
                            steps = []
                        else:
                            steps = range(NT2)
                        for step in steps:
                            t2 = step
                            if gi == 0 and _rep == 0:
                                if step == 0:
                                    # first slab in b-halves: the gate DRs
                                    # for bt 0/1 start ~0.7us sooner
                                    nc.sync.dma_start(
                                        xt_tiles[t2][:, :, :, :512],
                                        xc[t2][:, :, :, :512])
                                    nc.sync.dma_start(
                                        xt_tiles[t2][:, :, :, 512:],
                                        xc[t2][:, :, :, 512:])
                                else:
                                    nc.sync.dma_start(xt_tiles[t2][:], xc[t2])
                            for pe_, hv in prefetch_at.get(step, []):
                                if pe_ == "w2":
                                    nc.sync.dma_start(
                                        w2_sb[:],
                                        w2.rearrange("e h k -> h e k"))
                                elif pe_ == "smalls":
                                    _load_smalls()
                                elif hv is None:
                                    _w1_load(pe_)
                                else:
                                    _w1_load_half(pe_, hv)
                            if gi == 0:
                                for bt in range(NBT):
                                    po = pgate[bt // 2][:, (bt % 2) * BT:
                                                        (bt % 2 + 1) * BT]
                                    xt = xt_tiles[t2]
                                    bs = bass.ts(bt, BT)
                                    nc.tensor.matmul(
                                        po, gw_sb[:, 2 * t2:2 * t2 + 2, 0, :],
                                        xt[:, :, 1, bs],
                                        start=(t2 == 0 and bt % 2 == 0),
                                        stop=False, perf_mode=DR)
                                    nc.tensor.matmul(
                                        po, gw_sb[:, 2 * t2, :, :],
                                        xt[:, 0, :, bs], start=False,
                                        stop=False, perf_mode=DR)
                                    nc.tensor.matmul(
                                        po, gw_sb[:, 2 * t2 + 1, :, :],
                                        xt[:, 1, :, bs], start=False,
                                        stop=(t2 == NT2 - 1 and bt % 2 == 1),
                                        perf_mode=DR)
                            for e in grp:
                                if step < delay[e]:
                                    continue
                                ct2 = step - delay[e]
                                for bt in range(NBT):
                                    _l1_drs(e, ph1[e], ct2, bt,
                                            start=(step == delay[e]),
                                            stop=(step == NT2 - 1
                                                  and delay[e] == 0))

                        if gi == 0:
                            # softmax stage 1 issued before the catch-up so
                            # the ACT exp latency hides under catch-up DRs
                            expT, arT, recip = {}, {}, {}
                            for i in range(2):
                                expT[i] = stream.tile([E, 512], F32R,
                                                      tag="expT", bufs=2,
                                                      name=f"expT{i}")
                                nc.scalar.activation(expT[i][:],
                                                     pgate[i][0:E, :],
                                                     Exp, bias=gb_sb[:],
                                                     scale=S_L1)
                            # catch-up K-tile pairs skipped while W1 was in
                            # flight (same accumulation groups); each
                            # expert's silu issues as soon as it closes
                            for e in grp:
                                d = delay[e]
                                for j, ct2 in enumerate(range(NT2 - d, NT2)):
                                    for bt in range(NBT):
                                        _l1_drs(e, ph1[e], ct2, bt,
                                                start=False, stop=(j == d - 1))
                                _silu_h1(e, ph1[e])
                                if dbg and e == grp[0]:
                                    nc.sync.dma_start(dbg["h1"],
                                                      h1_tiles[e][:])
                            # softmax: gn8[e, b] = 8 * exp(z/512+gb) / sum_e
                            # expert-sum on the (idle) Pool engine, then DVE
                            # reciprocal * 8 and the normalize multiply
                            for i in range(2):
                                arT[i] = stream.tile([E, 512], F32R,
                                                     tag="arT", bufs=2,
                                                     name=f"arT{i}")
                                nc.gpsimd.partition_all_reduce(
                                    arT[i][:], expT[i][:], E,
                                    bass_isa.ReduceOp.add)
                            for i in range(2):
                                recip[i] = stream.tile([E, 512], F32R,
                                                       tag="recip", bufs=4,
                                                       name=f"recip{i}")
                                r8 = stream.tile([E, 512], F32R,
                                                 tag="recip", bufs=4,
                                                 name=f"r8_{i}")
                                with nc.allow_low_precision(
                                        reason="f32r softmax denom"):
                                    nc.vector.reciprocal(recip[i][:],
                                                         arT[i][:])
                                    nc.vector.tensor_scalar_mul(
                                        r8[:], recip[i][:], S_G)
                                nc.vector.tensor_mul(gn_sb[:, bass.ts(i, 512)],
                                                     expT[i][:], r8[:])
                            if dbg:
                                nc.sync.dma_start(dbg["expT"], expT[0][:])
                                nc.sync.dma_start(dbg["gn"], gn_sb[:])
                        elif gi < len(GROUPS) - 1:
                            for e in grp:
                                _silu_h1(e, ph1[e])

                        if gi == len(GROUPS) - 1:
                            # W3 streams in 4-dc slabs on the sync queue so
                            # the issue never serializes behind the scalar
                            # engine's activation backlog
                            w3_slabs = []
                            for sj in range(3):
                                w3s = stream.tile([128, 4, E, 2, H], F8,
                                                  tag="w3q", bufs=3,
                                                  name=f"w3s{sj}")
                                nc.sync.dma_start(w3s[:], w3c[sj])
                                w3_slabs.append(w3s)

                        # per-expert broadcast of this group's gn8 rows
                        # across 128 partitions on Pool (replaces the
                        # one-hot PE matmuls); Pool is otherwise idle
                        for e in grp:
                            for i in range(2):
                                pgb_tiles[e, i] = stream.tile(
                                    [128, 512], F32R, tag="pgb", bufs=6,
                                    name=f"pgb_{e % 3}_{i}")
                                nc.gpsimd.partition_broadcast(
                                    pgb_tiles[e, i][:],
                                    gn_sb[e:e + 1, bass.ts(i, 512)])

                        # L2 + gating + h2 quantization, batched by stage.
                        # i-major order: the b-half-0 chain for every expert
                        # completes first, so L3's late DRs (which consume
                        # h2c half 0 before half 1) start sooner.
                        ph2 = {}
                        for i in range(2):
                            for e in grp:
                                bs = bass.ts(i, 512)
                                ph2[e, i] = psA.tile([H, 512], F32, tag="acc",
                                                     bufs=6,
                                                     name=f"ph2_{e}_{i}")
                                nc.tensor.matmul(ph2[e, i][:], w2_sb[:, e, :],
                                                 h1_tiles[e][:, bs],
                                                 start=True, stop=True)
                        for i in range(2):
                            for e in grp:
                                h2t[e, i] = stream.tile([H, 512], F32,
                                                        tag="h2t", bufs=6,
                                                        name=f"h2t_{e}_{i}")
                                nc.scalar.activation(h2t[e, i][:],
                                                     ph2[e, i][:], Silu,
                                                     bias=b2_sb[:, e:e + 1],
                                                     scale=1.0)
                        for i in range(2):
                            for e in grp:
                                hc = h2c_tiles[e // 2]
                                ei = e % 2
                                bs = bass.ts(i, 512)
                                h2s8 = stream.tile([H, 512], F32, tag="h2s8",
                                                   bufs=4, name=f"h2s8_{e}_{i}")
                                with nc.allow_low_precision(
                                        reason="fp8 hi/lo split"):
                                    nc.vector.tensor_mul(h2s8[:], h2t[e, i][:],
                                                         pgb_tiles[e, i][:])
                                    nc.scalar.activation(hc[:, ei, 1, bs],
                                                         h2s8[:], Copy)
                                    nc.vector.tensor_sub(hc[:, ei, 0, bs],
                                                         h2s8[:],
                                                         hc[:, ei, 1, bs])

                    if dbg:
                        nc.sync.dma_start(dbg["h2c0"], h2c_tiles[0][:])
                    # ---- L3: outT[dc] = sum_e W3[e,dc].T @ h2s[e] ----
                    # Experts 0-5 are emitted before 6-7 in each tile so the
                    # PE can start L3 while the last group's h2 quantization
                    # chain (experts 6/7) is still draining on ACT/DVE.
                    # po tiles come from the same "acc" rotation as phase A
                    # (one shared psum pool -> no pool-close barrier).
                    # The first 4 dc open all 8 psum banks and emit their
                    # expert-0-5 DRs as one batch (~5us of PE work) before
                    # the first expert-6/7 DR, hiding the last group's
                    # gating-chain latency.
                    def _l3_early(po, w3t, dm, half):
                        for sub in range(2):
                            bt = half * 2 + sub
                            bs = bass.ts(bt, BT)
                            pslice = po[:, sub * BT:(sub + 1) * BT]
                            for j in range(3):
                                nc.tensor.matmul(
                                    pslice,
                                    w3t[:, dm, 2 * j:2 * j + 2, 0, :],
                                    h2c_tiles[j][:, :, 1, bs],
                                    start=(j == 0 and sub == 0),
                                    stop=False, perf_mode=DR)
                            for e in range(6):
                                nc.tensor.matmul(
                                    pslice, w3t[:, dm, e, :, :],
                                    h2c_tiles[e // 2][:, e % 2, :, bs],
                                    start=False, stop=False,
                                    perf_mode=DR)

                    def _l3_late(po, w3t, dm, half):
                        for sub in range(2):
                            bt = half * 2 + sub
                            bs = bass.ts(bt, BT)
                            pslice = po[:, sub * BT:(sub + 1) * BT]
                            nc.tensor.matmul(
                                pslice, w3t[:, dm, 6:8, 0, :],
                                h2c_tiles[3][:, :, 1, bs],
                                start=False, stop=False, perf_mode=DR)
                            for e in (6, 7):
                                nc.tensor.matmul(
                                    pslice, w3t[:, dm, e, :, :],
                                    h2c_tiles[3][:, e % 2, :, bs],
                                    start=False,
                                    stop=(e == 7 and sub == 1),
                                    perf_mode=DR)

                    po_head = {}
                    for dc in range(4):
                        for half in range(2):
                            if dc == 0:
                                po = psA.tile([128, 512], F32, tag="gate",
                                              bufs=2, name=f"pog{half}")
                            else:
                                po = psA.tile([128, 512], F32, tag="acc",
                                              bufs=6,
                                              name=f"po{(dc * 2 + half) % 6}")
                            po_head[dc, half] = po
                            _l3_early(po, w3_slabs[0], dc, half)

                    for dc in range(DCH):
                        if dc == 4:
                            w3s = stream.tile([128, 4, E, 2, H], F8,
                                              tag="w3q", bufs=3, name="w3s3")
                            nc.sync.dma_start(w3s[:], w3c[3])
                            w3_slabs.append(w3s)
                        w3t = w3_slabs[dc // 4]
                        dm = dc % 4
                        for half in range(2):
                            # dc0 rides the gate banks (idle since softmax)
                            # so L3 psum never waits on the last group's
                            # gating chain through the acc rotation
                            if dc < 4:
                                po = po_head[dc, half]
                            else:
                                po = psA.tile([128, 512], F32, tag="acc",
                                              bufs=6,
                                              name=f"po{(dc * 2 + half) % 6}")
                                _l3_early(po, w3t, dm, half)
                            _l3_late(po, w3t, dm, half)
                            # copy into the per-dc output staging tile;
                            # one DMA per dc (HWDGE descriptor-gen is the
                            # scarce resource, not bandwidth)
                            with nc.allow_low_precision(
                                    reason="bf16 output"):
                                if dc == DCH - 1:
                                    # tail: one engine per half (a split
                                    # within one tile serializes on the
                                    # tile's write tracking), DMA fired
                                    # immediately per half
                                    o_tl = stream.tile([128, 512], BF16,
                                                       tag="osbt", bufs=2,
                                                       name=f"ot{half}")
                                    if half == 0:
                                        nc.vector.tensor_scalar_mul(
                                            o_tl[:], po[:], S_L3)
                                    else:
                                        nc.scalar.activation(
                                            o_tl[:], po[:], Copy, scale=S_L3)
                                    nc.sync.dma_start(
                                        outT[dc * 128:(dc + 1) * 128,
                                             bass.ts(half, 512)], o_tl[:])
                                    continue
                                if half == 0:
                                    o_sb = stream.tile([128, Bs], BF16,
                                                       tag="osb", bufs=3,
                                                       name=f"osb{dc % 3}")
                                hs = bass.ts(half, 512)
                                if half == 0:
                                    nc.scalar.activation(o_sb[:, hs], po[:],
                                                         Copy, scale=S_L3)
                                else:
                                    nc.vector.tensor_scalar_mul(
                                        o_sb[:, hs], po[:], S_L3)
                            if half == 1:
                                nc.sync.dma_start(
                                    outT[dc * 128:(dc + 1) * 128, :],
                                    o_sb[:])
                                if dbg and dc == 0:
                                    nc.sync.dma_start(dbg["y0"], o_sb[:])

    nc.compile()
    return nc


_MODULE_CACHE = {}


def _get_module(reps=1):
    if reps not in _MODULE_CACHE:
        _MODULE_CACHE[reps] = _build_module(reps)
    return _MODULE_CACHE[reps]


def _hilo(a, scale):
    """Return (hi, lo) e4m3 arrays for a*scale, lo at the SAME scale."""
    s = np.float32(scale)
    hi = (a * s).astype(NPF8)
    lo = (a * s - hi.astype(np.float32)).astype(NPF8)
    return hi, lo


def _prep_shared(gate_w, gate_b, W1, b1, W2, b2, W3):
    # gate weights: [128, DCH, 2, E], pairs (hi, lo)
    gwr = np.zeros((128, DCH, 2 * E), dtype=np.float32)
    gwr[:, :, :E] = gate_w.reshape(DCH, 128, E).transpose(1, 0, 2)
    ghi, glo = _hilo(gwr, S_W)
    gwc = np.ascontiguousarray(np.stack([ghi, glo], axis=2))

    # W1: region A = [E, 128, 2*KW1, 2, H] (hi,lo pairs for K-tile pairs
    # 0..KW1-1), region B = [E, 128, DCH-2*KW1, H] hi only
    w1r = W1.reshape(E, DCH, 128, H).transpose(0, 2, 1, 3)
    w1hi, w1lo = _hilo(w1r, S_W)
    w1a = np.ascontiguousarray(
        np.stack([w1hi[:, :, :2 * KW1], w1lo[:, :, :2 * KW1]], axis=3))
    w1b = np.ascontiguousarray(w1hi[:, :, 2 * KW1:])

    # W3: [DCH, 128, E, 2, H]  (partition is the h contraction dim;
    # W3[e] is [H, D]: lhsT per dc = [128(h), 128(d)])
    w3r = W3.reshape(E, H, DCH, 128).transpose(2, 1, 0, 3)
    # w3r: [DCH, H(128 partitions), E, 128(d cols)]
    w3hi, w3lo = _hilo(w3r, S_W)
    w3c = np.stack([w3hi, w3lo], axis=3)          # [DCH, 128, E, 2, 128]
    w3c = np.ascontiguousarray(
        w3c.reshape(4, 4, 128, E, 2, 128).transpose(0, 2, 1, 3, 4, 5))

    return {
        "gwc": gwc, "gb": np.ascontiguousarray(gate_b.reshape(E, 1)),
        "w1a": w1a, "w1b": w1b, "b1t": np.ascontiguousarray(b1.T),
        "w2": np.ascontiguousarray(W2), "b2t": np.ascontiguousarray(b2.T),
        "w3c": w3c,
    }


def _prep_xc(x_slice):
    """x slice [Bs, D] -> [NT2, 128, 2, 2, Bs] fp8, pairs (lo, hi)."""
    xT = x_slice.T.reshape(DCH, 128, Bs)
    xhi, xlo = _hilo(xT, S_X)
    # [DCH, 128, 2(lo,hi), Bs] -> [NT2, 2, 128, 2, Bs] -> [NT2, 128, 2, 2, Bs]
    st = np.stack([xlo, xhi], axis=2).reshape(NT2, 2, 128, 2, Bs)
    return np.ascontiguousarray(st.transpose(0, 2, 1, 3, 4))


def kernel(x, gate_w, gate_b, W1, b1, W2, b2, W3, b3):
    x = np.asarray(x, dtype=np.float32)
    gate_w = np.asarray(gate_w, dtype=np.float32)
    gate_b = np.asarray(gate_b, dtype=np.float32)
    W1 = np.asarray(W1, dtype=np.float32)
    b1 = np.asarray(b1, dtype=np.float32)
    W2 = np.asarray(W2, dtype=np.float32)
    b2 = np.asarray(b2, dtype=np.float32)
    W3 = np.asarray(W3, dtype=np.float32)
    b3 = np.asarray(b3, dtype=np.float32)

    nc = _get_module(1)
    shared = _prep_shared(gate_w, gate_b, W1, b1, W2, b2, W3)
    in_maps = [{"xc": _prep_xc(x[i * Bs:(i + 1) * Bs, :]), **shared}
               for i in range(NCORES)]
    try:
        res = bass_utils.run_bass_kernel_spmd(
            nc, in_maps, core_ids=list(range(NCORES)))
    except Exception:
        # the axon-tunneled devices occasionally report a transient
        # NRT_EXEC_UNIT_UNRECOVERABLE; one retry after a pause clears it
        import time as _time
        _time.sleep(30)
        res = bass_utils.run_bass_kernel_spmd(
            nc, in_maps, core_ids=list(range(NCORES)))

    out = np.empty((B, D), dtype=np.float32)
    for i in range(NCORES):
        out[i * Bs:(i + 1) * Bs, :] = res.results[i]["outT"].T.astype(
            np.float32)

    if np.any(b3):
        # b3 contributes sum_e gates[b,e] * b3[e,d]; the device kernel skips
        # it (it is zero for this problem's inputs), so patch on host.
        logits = x @ gate_w + gate_b
        m = logits.max(axis=1, keepdims=True)
        p = np.exp(logits - m)
        gates = p / p.sum(axis=1, keepdims=True)
        out += gates @ b3
    return out
